# revision 2
# baseline (speedup 1.0000x reference)
"""Trainium2 Bass kernel for nn_LocalInteraction (SpookyNet-style local interaction).

Strategy (8 NeuronCores, SPMD):
  - Edges sharded by DESTINATION node: core c owns nodes [2000c, 2000c+2000)
    plus all edges whose receiver i lies there; 16 windows of 128 dest nodes,
    padded to a uniform per-window tile grid shared by all cores (one NEFF).
  - Node phase (replicated, feature-major): the three edge MLPs are computed
    per node (mlp(x_j) == mlp(x_tilde)[j]) over all 16000 nodes, transposed to
    node-major records [node, 384] in HBM.
  - Edge phase: records gathered by j via dma_gather (<=512 idx/call,
    edge-major); radial basis via exp/log formulation; scaled one-hots built
    as batched broadcast tensor_tensor ops (bf16); segment-sum is 4
    PSUM-accumulated matmuls per tile -> feature-major quants [f, n].
  - Window epilogue: P/D invariant contractions + final MLP -> output [f, n];
    host reassembles and transposes.
"""
import sys, os, math
if not any("trn_rl_repo" in p or "simrepo" in p for p in sys.path):
    sys.path.insert(0, "/opt/trn_rl_repo")
import numpy as np

import concourse.bass as bass
import concourse.bacc as bacc
import concourse.mybir as mybir
import concourse.tile as tile
from concourse.bass_utils import run_bass_kernel_spmd
from concourse.masks import make_identity
from concourse.tile import add_dep_helper

F32 = mybir.dt.float32
BF16 = mybir.dt.bfloat16
I16 = mybir.dt.int16
AF = mybir.ActivationFunctionType
ALU = mybir.AluOpType

N_NODES = 16000
FEAT = 128
GAMMA = 0.5
R_CUT = 5.0
N_CORES = 8
NPC = N_NODES // N_CORES          # 2000 nodes per core
NW = (NPC + 127) // 128           # 16 windows per core
NPAD = 16384
NCHUNK = NPAD // 512              # 32
LOCPAD = NW * 128                 # 2048
_BINOM = np.array([math.comb(15, k) for k in range(16)], np.float64)

LAST_EXEC_NS = None
_prog_cache = {}


def _build_program(n_t):
    nt_list = [x for x in n_t if not isinstance(x, str)]
    TT = sum(nt_list)
    starts = np.concatenate([[0], np.cumsum(nt_list)]).astype(int)
    ntmax = max(nt_list)
    stage = os.environ.get("KB_STAGE", "3")
    prec = os.environ.get("KB_PREC", "bf16")
    EDT = BF16 if prec == "bf16" else F32   # edge-data dtype (records/onehot/A)

    nc = bacc.Bacc("TRN2", target_bir_lowering=False, debug=False,
                   num_devices=N_CORES)
    for v in (1e-12,):
        t_ = nc.alloc_sbuf_tensor(f"const-float32-{v}", [128, 1], F32)
        nc.gpsimd.memset(t_.ap(), v)
        nc.const_aps.aps[(F32, v)] = t_.ap()
    nc.all_engine_barrier()

    CCOLS = 1920 + 30 + 176 + 384 + 512 + LOCPAD + TT
    xtT = nc.dram_tensor("xtT", [128, NPAD], F32, kind="ExternalInput")
    cstd = nc.dram_tensor("cstd", [128, CCOLS], F32, kind="ExternalInput")
    cstb = nc.dram_tensor("cstb", [128, 128 + TT], BF16, kind="ExternalInput")
    eidx = nc.dram_tensor("eidx", [128, TT * 8], I16, kind="ExternalInput")
    erij = nc.dram_tensor("erij", [128, TT * 3], F32, kind="ExternalInput")
    outT = nc.dram_tensor("outT", [128, LOCPAD], F32, kind="ExternalOutput")
    mrec = nc.dram_tensor("mrec", [NPAD, 384], EDT)

    def sv(idx, col):
        c = idx * 6 + col
        return svec_sb[:, c:c + 1]

    def wslice(idx, layer):
        k = 3 * idx + layer
        return wsb[:, k * 128:(k + 1) * 128]

    with tile.TileContext(nc) as tc:
        from contextlib import ExitStack
        es = ExitStack()
        cst = es.enter_context(tc.tile_pool(name="cst", bufs=1))

        cst_sb = cst.tile([128, CCOLS], F32)
        nc.sync.dma_start(out=cst_sb[:], in_=cstd[:])
        o = 0
        wsb = cst_sb[:, o:o + 1920]; o += 1920
        svec_sb = cst_sb[:, o:o + 30]; o += 30
        crow_sb = cst_sb[:, o:o + 176]; o += 176
        grep_sb = cst_sb[:, o:o + 384]; o += 384
        pdm_sb = cst_sb[:, o:o + 512]; o += 512
        xtl_sb = cst_sb[:, o:o + LOCPAD]; o += LOCPAD
        erel_sb = cst_sb[:, o:o + TT]; o += TT
        cstb_sb = cst.tile([128, 128 + TT], BF16)
        nc.sync.dma_start(out=cstb_sb[:], in_=cstb[:])
        eidx_sb = cst.tile([128, TT * 8], I16)
        nc.sync.dma_start(out=eidx_sb[:], in_=eidx[:])
        ident = cst.tile([128, 128], F32)
        make_identity(nc, ident[:])
        cterm = cst.tile([128, LOCPAD], F32)

        if EDT == BF16:
            iotae, erele = cstb_sb[:, 0:128], cstb_sb[:, 128:128 + TT]
        else:
            iotae, erele = crow_sb[:, 0:128], erel_sb
        gk_b = crow_sb[:, 128:144].rearrange("p (o k) -> p o k", o=1)
        k15_b = crow_sb[:, 144:160].rearrange("p (o k) -> p o k", o=1)
        lnb_b = crow_sb[:, 160:176].rearrange("p (o k) -> p o k", o=1)

        def resmlp_chunk(idx, x_ap, ncols, sbp, psp, out_ap, out_dve=False):
            sw = sbp.tile([128, ncols], F32, tag="sw", bufs=3, name="sw")
            nc.scalar.activation(sw[:], x_ap, AF.Silu, scale=sv(idx, 0))
            h1 = psp.tile([128, ncols], F32, space="PSUM", tag="h", bufs=6,
                          name="h1")
            nc.tensor.matmul(out=h1[:], lhsT=wslice(idx, 0), rhs=sw[:],
                             start=True, stop=True)
            sw2 = sbp.tile([128, ncols], F32, tag="sw2", bufs=3, name="sw2")
            nc.scalar.activation(sw2[:], h1[:], AF.Silu, scale=sv(idx, 1),
                                 bias=sv(idx, 3))
            h2 = psp.tile([128, ncols], F32, space="PSUM", tag="h", bufs=6,
                          name="h2")
            nc.tensor.matmul(out=h2[:], lhsT=wslice(idx, 1), rhs=sw2[:],
                             start=True, stop=True)
            r = sbp.tile([128, ncols], F32, tag="r", bufs=3, name="r")
            nc.vector.scalar_tensor_tensor(out=r[:], in0=h2[:], scalar=sv(idx, 4),
                                           in1=x_ap, op0=ALU.add, op1=ALU.add)
            sw3 = sbp.tile([128, ncols], F32, tag="sw3", bufs=3, name="sw3")
            nc.scalar.activation(sw3[:], r[:], AF.Silu, scale=sv(idx, 2))
            h3 = psp.tile([128, ncols], F32, space="PSUM", tag="h", bufs=6,
                          name="h3")
            nc.tensor.matmul(out=h3[:], lhsT=wslice(idx, 2), rhs=sw3[:],
                             start=True, stop=True)
            if out_dve:
                nc.vector.tensor_scalar(out=out_ap, in0=h3[:],
                                        scalar1=sv(idx, 5), scalar2=None,
                                        op0=ALU.add)
            else:
                nc.scalar.activation(out_ap, h3[:], AF.Identity,
                                     bias=sv(idx, 5))
            return h3

        # ------------------------------------------------------------------
        # node phase
        # ------------------------------------------------------------------
        stage_dmas = []
        with (
            tc.tile_pool(name="xt", bufs=1) as xtp,
            tc.tile_pool(name="nod", bufs=3) as nod,
            tc.tile_pool(name="stg", bufs=3) as stg,
            tc.tile_pool(name="nps", bufs=6, space="PSUM") as nps,
            tc.tile_pool(name="tps", bufs=2, space="PSUM") as tps,
        ):
            xt_sb = xtp.tile([128, NPAD], F32)
            nc.sync.dma_start(out=xt_sb[:], in_=xtT[:])
            for ch in range(NCHUNK):
                x_ap = xt_sb[:, ch * 512:(ch + 1) * 512]
                stage_t = stg.tile([128, 4, 384], EDT, tag="stage", name="stage")
                for idx in (1, 2, 3):
                    m_sb = nod.tile([128, 512], F32, tag="msb", name="msb")
                    resmlp_chunk(idx, x_ap, 512, nod, nps, m_sb[:],
                                 out_dve=True)
                    for b in range(4):
                        tp = tps.tile([128, 128], F32, space="PSUM", tag="tp",
                                      name="tp")
                        nc.tensor.transpose(out=tp[:],
                                            in_=m_sb[:, b * 128:(b + 1) * 128],
                                            identity=ident[:])
                        dst = stage_t[:, b, (idx - 1) * 128: idx * 128]
                        if b < 2:
                            nc.scalar.copy(dst, tp[:])
                        else:
                            nc.vector.tensor_copy(dst, tp[:])
                dst = mrec[ch * 512:(ch + 1) * 512, :].rearrange(
                    "(c p) f -> p c f", p=128)
                dma = nc.sync.dma_start(out=dst, in_=stage_t[:])
                stage_dmas.append(dma)
            for ch in range(LOCPAD // 512):
                x_ap = xtl_sb[:, ch * 512:(ch + 1) * 512]
                resmlp_chunk(0, x_ap, 512, nod, nps,
                             cterm[:, ch * 512:(ch + 1) * 512])

        if stage == "1":
            nc.sync.dma_start(out=outT[:], in_=cterm[:])
        # ------------------------------------------------------------------
        # edge phase
        # ------------------------------------------------------------------
        if stage != "1":
          with (
              tc.tile_pool(name="rec", bufs=2) as recp,
              tc.tile_pool(name="rad", bufs=2) as radp,
              tc.tile_pool(name="sc", bufs=2) as scp,
              tc.tile_pool(name="ohp", bufs=2) as ohp,
              tc.tile_pool(name="ap_", bufs=2) as app,
              tc.tile_pool(name="epi", bufs=2) as epip,
              tc.tile_pool(name="acc_ps", bufs=1, space="PSUM") as accp,
              tc.tile_pool(name="rg_ps", bufs=1, space="PSUM") as rgp,
              tc.tile_pool(name="scr_ps", bufs=2, space="PSUM") as scrp,
          ):
            for w in range(NW):
                nt = nt_list[w]
                T0 = int(starts[w])

                rec = recp.tile([128, ntmax, 384], EDT, tag="rec", name="rec")
                for g4 in range((nt + 3) // 4):
                    gsz4 = min(4, nt - g4 * 4)
                    gT = T0 + g4 * 4
                    g = nc.gpsimd.dma_gather(
                        rec[:, g4 * 4:g4 * 4 + gsz4, :], mrec[:],
                        eidx_sb[:, gT * 8:(gT + gsz4) * 8],
                        gsz4 * 128, gsz4 * 128, 384)
                    for sd in stage_dmas:
                        add_dep_helper(g.ins, sd.ins, reason="rec before gather")

                rij = radp.tile([128, ntmax, 3], F32, tag="rij", name="rij")
                nc.sync.dma_start(out=rij[:, 0:nt, :],
                                  in_=erij[:, T0 * 3:(T0 + nt) * 3]
                                  .rearrange("p (t c) -> p t c", c=3))
                rijv = rij[:, 0:nt, :]

                def sctile(tag):
                    t = scp.tile([128, ntmax, 1], F32, tag=tag, name=tag)
                    return t[:, 0:nt, :]

                sq = scp.tile([128, ntmax, 3], F32, tag="sq", name="sq")[:, 0:nt, :]
                nc.vector.tensor_tensor(out=sq, in0=rijv, in1=rijv, op=ALU.mult)
                d2 = sctile("d2")
                nc.vector.tensor_reduce(out=d2, in_=sq, axis=mybir.AxisListType.X,
                                        op=ALU.add)
                d = sctile("d")
                nc.scalar.activation(d, d2, AF.Sqrt, bias=1e-12)
                inv_d = sctile("ivd")
                nc.vector.reciprocal(inv_d, d)
                u = scp.tile([128, ntmax, 3], F32, tag="u", name="u")[:, 0:nt, :]
                nc.vector.tensor_tensor(out=u, in0=rijv,
                                        in1=inv_d.to_broadcast([128, nt, 3]),
                                        op=ALU.mult)
                usq = scp.tile([128, ntmax, 3], F32, tag="usq", name="usq")[:, 0:nt, :]
                nc.vector.tensor_tensor(out=usq, in0=u, in1=u, op=ALU.mult)
                y2 = scp.tile([128, ntmax, 5], F32, tag="y2", name="y2")[:, 0:nt, :]
                nc.vector.tensor_tensor(out=y2[:, :, 0:1], in0=u[:, :, 0:1],
                                        in1=u[:, :, 1:2], op=ALU.mult)
                nc.vector.tensor_tensor(out=y2[:, :, 1:2], in0=u[:, :, 0:1],
                                        in1=u[:, :, 2:3], op=ALU.mult)
                nc.vector.tensor_tensor(out=y2[:, :, 2:3], in0=u[:, :, 1:2],
                                        in1=u[:, :, 2:3], op=ALU.mult)
                nc.vector.tensor_tensor(out=y2[:, :, 3:4], in0=usq[:, :, 0:1],
                                        in1=usq[:, :, 1:2], op=ALU.subtract)
                nc.vector.tensor_scalar(out=y2[:, :, 4:5], in0=usq[:, :, 2:3],
                                        scalar1=3.0, scalar2=-1.0,
                                        op0=ALU.mult, op1=ALU.add)
                # bf16 copies of u / y2 for one-hot scaling
                ub = scp.tile([128, ntmax, 3], EDT, tag="ub", name="ub")
                nc.vector.tensor_copy(ub[:, 0:nt, :], u)
                y2b = scp.tile([128, ntmax, 5], EDT, tag="y2b", name="y2b")
                nc.vector.tensor_copy(y2b[:, 0:nt, :], y2)
                rho = sctile("rho")
                nc.scalar.activation(rho, d, AF.Exp, scale=-GAMMA)
                om = sctile("om")
                nc.vector.tensor_scalar(out=om, in0=rho, scalar1=-1.0,
                                        scalar2=1.0, op0=ALU.mult, op1=ALU.add)
                nc.vector.tensor_scalar(out=om, in0=om, scalar1=1e-38,
                                        scalar2=None, op0=ALU.max)
                lg = sctile("lg")
                nc.scalar.activation(lg, om, AF.Ln)
                den = sctile("den")
                nc.vector.tensor_scalar(out=den, in0=d2, scalar1=-1.0,
                                        scalar2=R_CUT * R_CUT,
                                        op0=ALU.mult, op1=ALU.add)
                rden = sctile("rdn")
                nc.vector.reciprocal(rden, den)
                mme = sctile("mme")
                nc.vector.tensor_tensor(out=mme, in0=d2, in1=rden, op=ALU.mult)
                msk = sctile("msk")
                nc.vector.tensor_scalar(out=msk, in0=d, scalar1=R_CUT,
                                        scalar2=None, op0=ALU.is_ge)
                arge = sctile("age")
                nc.vector.scalar_tensor_tensor(out=arge, in0=msk, scalar=1e30,
                                               in1=mme, op0=ALU.mult, op1=ALU.add)
                arg = radp.tile([128, ntmax, 32], F32, tag="arg", name="arg")
                argv = arg[:, 0:nt, 0:16]
                nc.vector.tensor_tensor(out=argv,
                                        in0=d.to_broadcast([128, nt, 16]),
                                        in1=gk_b.to_broadcast([128, nt, 16]),
                                        op=ALU.mult)
                tmp16 = radp.tile([128, ntmax, 16], F32, tag="t16", name="t16")
                nc.vector.tensor_tensor(out=tmp16[:, 0:nt, :],
                                        in0=lg.to_broadcast([128, nt, 16]),
                                        in1=k15_b.to_broadcast([128, nt, 16]),
                                        op=ALU.mult)
                nc.vector.tensor_tensor(out=argv, in0=argv,
                                        in1=tmp16[:, 0:nt, :], op=ALU.add)
                nc.vector.tensor_tensor(out=argv, in0=argv,
                                        in1=arge.to_broadcast([128, nt, 16]),
                                        op=ALU.subtract)
                nc.vector.tensor_tensor(out=argv, in0=argv,
                                        in1=lnb_b.to_broadcast([128, nt, 16]),
                                        op=ALU.add)
                bern = radp.tile([128, ntmax, 32], F32, tag="bern", name="bern")
                nc.vector.memset(bern[:, 0:nt, 16:32], 0.0)
                nc.scalar.activation(bern[:, 0:nt, 0:16], argv, AF.Exp)

                acc_s = accp.tile([128, 128], F32, space="PSUM", tag="acc_s")
                acc_p = accp.tile([128, 384], F32, space="PSUM", tag="acc_p")
                acc_d = accp.tile([128, 640], F32, space="PSUM", tag="acc_d")

                for gix in range((nt + 3) // 4):
                    gsz = min(4, nt - gix * 4)
                    tp = scrp.tile([128, 512], F32, space="PSUM", tag="scr",
                                   name="tscr")
                    nc.tensor.transpose(
                        out=tp[0:32 * gsz, 0:128],
                        in_=bern[:, gix * 4: gix * 4 + gsz, :],
                        identity=ident[:])
                    radT = radp.tile([128, 128], F32, tag="radT", name="radT")
                    nc.scalar.copy(radT[0:32 * gsz, :], tp[0:32 * gsz, 0:128])

                    # batched one-hot construction for the group (bf16)
                    ohb = ohp.tile([128, 4, 9, 128], EDT, tag="oh", name="oh")
                    irelb = erele[:, T0 + gix * 4: T0 + gix * 4 + gsz] \
                        .rearrange("p (t o) -> p t o", o=1)
                    nc.vector.tensor_tensor(
                        out=ohb[:, 0:gsz, 0, :],
                        in0=iotae.rearrange("p (o n) -> p o n", o=1)
                        .to_broadcast([128, gsz, 128]),
                        in1=irelb.to_broadcast([128, gsz, 128]),
                        op=ALU.is_equal)
                    ubg = ub[:, gix * 4:gix * 4 + gsz, :] \
                        .rearrange("p t (c o) -> p t c o", o=1)
                    nc.vector.tensor_tensor(
                        out=ohb[:, 0:gsz, 1:4, :],
                        in0=ohb[:, 0:gsz, 0:1, :].to_broadcast([128, gsz, 3, 128]),
                        in1=ubg.to_broadcast([128, gsz, 3, 128]),
                        op=ALU.mult)
                    y2g = y2b[:, gix * 4:gix * 4 + gsz, :] \
                        .rearrange("p t (c o) -> p t c o", o=1)
                    nc.vector.tensor_tensor(
                        out=ohb[:, 0:gsz, 4:9, :],
                        in0=ohb[:, 0:gsz, 0:1, :].to_broadcast([128, gsz, 5, 128]),
                        in1=y2g.to_broadcast([128, gsz, 5, 128]),
                        op=ALU.mult)

                    for pair in range((gsz + 1) // 2):
                        psz = min(2, gsz - pair * 2)
                        radG2 = rgp.tile([128, 2, 512], F32, space="PSUM",
                                         tag="rg", name="rg")
                        for k in range(psz):
                            q = pair * 2 + k
                            nc.tensor.matmul(
                                out=radG2[:, k, 0:384],
                                lhsT=radT[32 * q:32 * q + 32, :],
                                rhs=grep_sb[32 * q:32 * q + 32, :],
                                start=True, stop=True,
                                tile_position=(32 * q, 0))
                        rgsb = app.tile([128, 2, 384], EDT, tag="rgsb",
                                        name="rgsb")
                        nc.scalar.copy(rgsb[:, 0:psz, :],
                                       radG2[:, 0:psz, 0:384])
                        A2 = app.tile([128, 2, 384], EDT, tag="A", name="A")
                        t0_ = gix * 4 + pair * 2
                        nc.vector.tensor_tensor(
                            out=A2[:, 0:psz, :], in0=rec[:, t0_:t0_ + psz, :],
                            in1=rgsb[:, 0:psz, :], op=ALU.mult)
                        for k in range(psz):
                            t = t0_ + k
                            st, sp = (t == 0), (t == nt - 1)
                            A_ = A2[:, k, :]
                            oh_ = ohb[:, t - gix * 4, :, :]
                            nc.tensor.matmul(out=acc_s[:], lhsT=A_[:, 0:128],
                                             rhs=oh_[:, 0, :], start=st, stop=sp)
                            nc.tensor.matmul(out=acc_p[:], lhsT=A_[:, 128:256],
                                             rhs=oh_[:, 1:4, :], start=st,
                                             stop=sp)
                            nc.tensor.matmul(out=acc_d[:, 0:512],
                                             lhsT=A_[:, 256:384],
                                             rhs=oh_[:, 4:8, :], start=st,
                                             stop=sp)
                            nc.tensor.matmul(out=acc_d[:, 512:640],
                                             lhsT=A_[:, 256:384],
                                             rhs=oh_[:, 8, :], start=st, stop=sp)

                # ---- window epilogue ----
                qsb = epip.tile([128, 1152], F32, tag="qsb", name="qsb")
                nc.scalar.copy(qsb[:, 0:128], acc_s[:])
                nc.scalar.copy(qsb[:, 128:512], acc_p[:])
                nc.scalar.copy(qsb[:, 512:1152], acc_d[:])
                inp = epip.tile([128, 128], F32, tag="inp", name="inp")
                nc.vector.tensor_tensor(out=inp[:], in0=acc_s[:],
                                        in1=cterm[:, w * 128:(w + 1) * 128],
                                        op=ALU.add)
                t1 = scrp.tile([128, 512], F32, space="PSUM", tag="scr", name="t1")
                t2 = scrp.tile([128, 512], F32, space="PSUM", tag="scr", name="t2")
                nc.tensor.matmul(out=t1[:, 0:384], lhsT=pdm_sb[:, 0:128],
                                 rhs=qsb[:, 128:512], start=True, stop=True)
                nc.tensor.matmul(out=t2[:, 0:384], lhsT=pdm_sb[:, 128:256],
                                 rhs=qsb[:, 128:512], start=True, stop=True)
                t1sb = epip.tile([128, 512], F32, tag="t1sb", name="t1sb")
                nc.scalar.copy(t1sb[:, 0:384], t1[:, 0:384])
                pp = epip.tile([128, 128, 5], F32, tag="pp", name="pp")
                nc.vector.tensor_tensor(
                    out=pp[:, :, 0:3].rearrange("p n c -> p c n"),
                    in0=t1sb[:, 0:384].rearrange("p (c n) -> p c n", n=128),
                    in1=t2[:, 0:384].rearrange("p (c n) -> p c n", n=128),
                    op=ALU.mult)
                red = epip.tile([128, 128], F32, tag="red", name="red")
                nc.vector.tensor_reduce(out=red[:], in_=pp[:, :, 0:3],
                                        axis=mybir.AxisListType.X, op=ALU.add)
                nc.vector.tensor_tensor(out=inp[:], in0=inp[:], in1=red[:],
                                        op=ALU.add)
                t1d = scrp.tile([128, 512], F32, space="PSUM", tag="scr", name="t1d")
                t2d = scrp.tile([128, 512], F32, space="PSUM", tag="scr", name="t2d")
                nc.tensor.matmul(out=t1d[:, 0:512], lhsT=pdm_sb[:, 256:384],
                                 rhs=qsb[:, 512:1024], start=True, stop=True)
                nc.tensor.matmul(out=t2d[:, 0:512], lhsT=pdm_sb[:, 384:512],
                                 rhs=qsb[:, 512:1024], start=True, stop=True)
                t1dsb = epip.tile([128, 512], F32, tag="t1sb", name="t1dsb")
                nc.scalar.copy(t1dsb[:], t1d[:])
                ppd = epip.tile([128, 128, 5], F32, tag="pp", name="ppd")
                nc.vector.tensor_tensor(
                    out=ppd[:, :, 0:4].rearrange("p n c -> p c n"),
                    in0=t1dsb[:].rearrange("p (c n) -> p c n", n=128),
                    in1=t2d[:].rearrange("p (c n) -> p c n", n=128),
                    op=ALU.mult)
                t1e = scrp.tile([128, 512], F32, space="PSUM", tag="scr", name="t1e")
                t2e = scrp.tile([128, 512], F32, space="PSUM", tag="scr", name="t2e")
                nc.tensor.matmul(out=t1e[:, 0:128], lhsT=pdm_sb[:, 256:384],
                                 rhs=qsb[:, 1024:1152], start=True, stop=True)
                nc.tensor.matmul(out=t2e[:, 0:128], lhsT=pdm_sb[:, 384:512],
                                 rhs=qsb[:, 1024:1152], start=True, stop=True)
                t1esb = epip.tile([128, 128], F32, tag="t1esb", name="t1esb")
                nc.scalar.copy(t1esb[:], t1e[:, 0:128])
                nc.vector.tensor_tensor(
                    out=ppd[:, :, 4:5].rearrange("p n c -> p c n"),
                    in0=t1esb[:].rearrange("p (c n) -> p c n", n=128),
                    in1=t2e[:, 0:128].rearrange("p (c n) -> p c n", n=128),
                    op=ALU.mult)
                redd = epip.tile([128, 128], F32, tag="red", name="redd")
                nc.vector.tensor_reduce(out=redd[:], in_=ppd[:],
                                        axis=mybir.AxisListType.X, op=ALU.add)
                nc.vector.tensor_tensor(out=inp[:], in0=inp[:], in1=redd[:],
                                        op=ALU.add)
                # final mlp (idx 4)
                sw = epip.tile([128, 128], F32, tag="fsw", name="fsw")
                nc.scalar.activation(sw[:], inp[:], AF.Silu, scale=sv(4, 0))
                h1 = scrp.tile([128, 512], F32, space="PSUM", tag="scr", name="fh1")
                nc.tensor.matmul(out=h1[:, 0:128], lhsT=wslice(4, 0), rhs=sw[:],
                                 start=True, stop=True)
                sw2 = epip.tile([128, 128], F32, tag="fsw2", name="fsw2")
                nc.scalar.activation(sw2[:], h1[:, 0:128], AF.Silu,
                                     scale=sv(4, 1), bias=sv(4, 3))
                h2 = scrp.tile([128, 512], F32, space="PSUM", tag="scr", name="fh2")
                nc.tensor.matmul(out=h2[:, 0:128], lhsT=wslice(4, 1), rhs=sw2[:],
                                 start=True, stop=True)
                r4 = epip.tile([128, 128], F32, tag="fr", name="fr")
                nc.vector.scalar_tensor_tensor(out=r4[:], in0=h2[:, 0:128],
                                               scalar=sv(4, 4), in1=inp[:],
                                               op0=ALU.add, op1=ALU.add)
                sw3 = epip.tile([128, 128], F32, tag="fsw3", name="fsw3")
                nc.scalar.activation(sw3[:], r4[:], AF.Silu, scale=sv(4, 2))
                h3 = scrp.tile([128, 512], F32, space="PSUM", tag="scr", name="fh3")
                nc.tensor.matmul(out=h3[:, 0:128], lhsT=wslice(4, 2), rhs=sw3[:],
                                 start=True, stop=True)
                outw = epip.tile([128, 128], F32, tag="outw", name="outw")
                nc.scalar.activation(outw[:], h3[:, 0:128], AF.Identity,
                                     bias=sv(4, 5))
                nc.sync.dma_start(out=outT[:, w * 128:(w + 1) * 128],
                                  in_=outw[:])
        es.close()
    nc.compile()
    return nc


# ----------------------------------------------------------------------------
# host side
# ----------------------------------------------------------------------------

def _prep_host(xyz, x_tilde, nbrs, W1, b1, W2, b2, W3, b3, alpha, beta,
               G_s, G_p, G_d, P_1, P_2, D_1, D_2):
    xyz = np.asarray(xyz, np.float32)
    x_tilde = np.asarray(x_tilde, np.float32)
    nbrs = np.asarray(nbrs)
    i = nbrs[:, 0].astype(np.int64)
    j = nbrs[:, 1].astype(np.int64)
    E = i.shape[0]

    r_ij = (xyz[j] - xyz[i]).astype(np.float32)

    core = i // NPC
    iloc = i - core * NPC
    w = iloc >> 7
    irel = (iloc & 127).astype(np.float32)
    key = core * NW + w
    order = np.argsort(key, kind="stable")
    cnt = np.bincount(key, minlength=N_CORES * NW).reshape(N_CORES, NW)
    n_t = np.maximum(1, -(-cnt.max(axis=0) // 128)).astype(int)
    TT = int(n_t.sum())
    starts = np.concatenate([[0], np.cumsum(n_t)]).astype(int)
    EPAD = TT * 128

    j_pad = np.zeros((N_CORES, EPAD), np.int64)
    irel_pad = np.full((N_CORES, EPAD), 200.0, np.float32)
    rij_pad = np.zeros((N_CORES, EPAD, 3), np.float32)

    cnt_flat = cnt.reshape(-1)
    grp_start = np.concatenate([[0], np.cumsum(cnt_flat)])[:-1]
    pos_in_grp = np.arange(E) - np.repeat(grp_start, cnt_flat)
    core_s = core[order]
    w_s = w[order]
    slot = starts[w_s] * 128 + pos_in_grp
    j_pad[core_s, slot] = j[order]
    irel_pad[core_s, slot] = irel[order]
    rij_pad[core_s, slot] = r_ij[order]

    eidx = np.zeros((N_CORES, 128, TT * 8), np.int16)
    for wi in range(NW):
        nt = int(n_t[wi]); base = int(starts[wi])
        jw = j_pad[:, base * 128:(base + nt) * 128]
        c = np.arange(nt * 8)
        t, q = c // 8, c % 8
        r16 = np.arange(16)
        e_ix = t[None, :] * 128 + r16[:, None] + 16 * q[None, :]
        blk = jw[:, e_ix].astype(np.int16)
        eidx[:, :, base * 8:(base + nt) * 8] = np.tile(blk, (1, 8, 1))

    erel = irel_pad.reshape(N_CORES, TT, 128).transpose(0, 2, 1).copy()
    erij = rij_pad.reshape(N_CORES, TT, 128, 3).transpose(0, 2, 1, 3) \
        .reshape(N_CORES, 128, TT * 3).copy()

    alpha = np.asarray(alpha, np.float64)
    beta = np.asarray(beta, np.float64)
    W1 = np.asarray(W1, np.float64); W2 = np.asarray(W2, np.float64)
    W3 = np.asarray(W3, np.float64)
    b1 = np.asarray(b1, np.float64); b2 = np.asarray(b2, np.float64)
    b3 = np.asarray(b3, np.float64)
    assert np.all(np.abs(beta) > 1e-6), "beta==0 unsupported by silu fold"

    wmats, svcols = [], np.zeros((128, 30), np.float32)
    for idx in range(5):
        a, b = alpha[idx], beta[idx]
        wmats += [(a[0] / b[0])[:, None] * W1[idx],
                  (a[1] / b[1])[:, None] * W2[idx],
                  (a[2] / b[2])[:, None] * W3[idx]]
        svcols[:, idx * 6 + 0] = b[0]
        svcols[:, idx * 6 + 1] = b[1]
        svcols[:, idx * 6 + 2] = b[2]
        svcols[:, idx * 6 + 3] = b[1] * b1[idx]
        svcols[:, idx * 6 + 4] = b2[idx]
        svcols[:, idx * 6 + 5] = b3[idx]
    wst = np.stack(wmats).astype(np.float32).transpose(1, 0, 2) \
        .reshape(128, 15 * 128).copy()

    crow = np.zeros((128, 176), np.float32)
    crow[:, 0:128] = np.arange(128, dtype=np.float32)[None, :]
    ks = np.arange(16, dtype=np.float64)
    crow[:, 128:144] = (-GAMMA * ks)[None, :]
    crow[:, 144:160] = (15.0 - ks)[None, :]
    crow[:, 160:176] = np.log(_BINOM)[None, :]

    grep_np = np.zeros((128, 384), np.float32)
    for q in range(4):
        for X, G in enumerate([G_s, G_p, G_d]):
            grep_np[32 * q:32 * q + 16, X * 128:(X + 1) * 128] = \
                np.asarray(G, np.float32).T
    pdm_np = np.concatenate([np.asarray(M, np.float32).T for M in
                             (P_1, P_2, D_1, D_2)], axis=1).copy()

    xtT_np = np.zeros((128, NPAD), np.float32)
    xtT_np[:, :N_NODES] = x_tilde.T
    xtl_np = np.zeros((N_CORES, 128, LOCPAD), np.float32)
    for cix in range(N_CORES):
        xtl_np[cix, :, :NPC] = x_tilde[cix * NPC:(cix + 1) * NPC].T

    import ml_dtypes
    iota_bf = np.arange(128, dtype=np.float32)[None, :].repeat(128, 0)
    in_maps = []
    for cix in range(N_CORES):
        cstd = np.concatenate(
            [wst, svcols, crow, grep_np, pdm_np, xtl_np[cix], erel[cix]],
            axis=1).astype(np.float32)
        cstb = np.concatenate([iota_bf, erel[cix]], axis=1) \
            .astype(ml_dtypes.bfloat16)
        in_maps.append({
            "xtT": xtT_np, "cstd": cstd, "cstb": cstb,
            "eidx": eidx[cix], "erij": erij[cix],
        })
    return tuple(int(x) for x in n_t), in_maps


def kernel(**inputs) -> np.ndarray:
    global LAST_EXEC_NS
    n_t, in_maps = _prep_host(**inputs)
    key = n_t + (os.environ.get('KB_STAGE', '3'), os.environ.get('KB_PREC', 'bf16'))
    if key not in _prog_cache:
        _prog_cache[key] = _build_program(key)
    nc = _prog_cache[key]

    trace = os.environ.get("KBENCH_TRACE", "0") == "1"
    res = run_bass_kernel_spmd(nc, in_maps, core_ids=list(range(N_CORES)),
                               trace=trace)
    if trace:
        LAST_EXEC_NS = res.exec_time_ns
        global LAST_RESULT
        LAST_RESULT = res
        if res.instructions_and_trace:
            print(f"trace path: {res.instructions_and_trace[1]}")

    out = np.empty((N_NODES, FEAT), np.float32)
    for cix in range(N_CORES):
        out[cix * NPC:(cix + 1) * NPC] = res.results[cix]["outT"][:, :NPC].T
    return out



# revision 20
# speedup vs baseline: 2.4379x; 2.4379x over previous
"""Trainium2 Bass kernel for nn_LocalInteraction (SpookyNet-style local interaction).

Strategy (8 NeuronCores, SPMD):
  - Edges sharded by DESTINATION node: core c owns nodes [2000c, 2000c+2000)
    plus all edges whose receiver i lies there; NW windows of W dest nodes,
    padded to a uniform per-window tile grid shared by all cores (one NEFF).
  - Node phase (replicated, feature-major): the three edge MLPs are computed
    per node (mlp(x_j) == mlp(x_tilde)[j]) over all 16000 nodes, transposed to
    node-major records [node, 384] in HBM.
  - Radial basis computed in ONE batched pass over all edge tiles (exp/log
    formulation, single ACT table set) -> bern/u/Y2 in bf16.
  - Edge phase: records gathered by j via one dma_gather per window
    (edge-major); scaled one-hots built as batched broadcast tensor_tensor
    ops (bf16); segment-sum is 4 PSUM-accumulated matmuls per tile.
  - Window epilogue: P/D invariant contractions accumulate into an SBUF
    inp buffer; the final MLP runs ONCE batched over all local nodes.
"""
import sys, os, math
if not any("trn_rl_repo" in p or "simrepo" in p for p in sys.path):
    sys.path.insert(0, "/opt/trn_rl_repo")
import numpy as np

import concourse.bass as bass
import concourse.bacc as bacc
import concourse.mybir as mybir
import concourse.tile as tile
from concourse.bass_utils import run_bass_kernel_spmd
from concourse.masks import make_identity
from concourse.tile import add_dep_helper

F32 = mybir.dt.float32
BF16 = mybir.dt.bfloat16
I16 = mybir.dt.int16
AF = mybir.ActivationFunctionType
ALU = mybir.AluOpType

N_NODES = 16000
FEAT = 128
GAMMA = 0.5
R_CUT = 5.0
N_CORES = 8
NPC = N_NODES // N_CORES          # 2000 nodes per core
W = int(os.environ.get("KB_W", "64"))   # dest-window width
NW = (NPC + W - 1) // W           # windows per core
NPAD = 16384
SHARD = NPAD // N_CORES           # 2048 record-nodes per core
NCHUNK = SHARD // 512             # 4 chunks per core (sharded node phase)
LOCPAD = NW * W                   # padded local node count
_BINOM = np.array([math.comb(15, k) for k in range(16)], np.float64)

LAST_EXEC_NS = None
LAST_RESULT = None
_prog_cache = {}


def _build_program(n_t):
    nt_list = [x for x in n_t if not isinstance(x, str)]
    TT = sum(nt_list)
    starts = np.concatenate([[0], np.cumsum(nt_list)]).astype(int)
    ntmax = max(nt_list)

    nc = bacc.Bacc("TRN2", target_bir_lowering=False, debug=False,
                   num_devices=N_CORES)
    for v in (1e-12,):
        t_ = nc.alloc_sbuf_tensor(f"const-float32-{v}", [128, 1], F32)
        nc.gpsimd.memset(t_.ap(), v)
        nc.const_aps.aps[(F32, v)] = t_.ap()
    nc.all_engine_barrier()

    LCP = LOCPAD  # padded local columns (inp/cterm/out)
    CCOLS = 1920 + 30 + 176 + 384 + 512 + LCP + TT
    xtT = nc.dram_tensor("xtT", [128, SHARD], F32, kind="ExternalInput")
    cstd = nc.dram_tensor("cstd", [128, CCOLS], F32, kind="ExternalInput")
    cstb = nc.dram_tensor("cstb", [128, 128 + TT], BF16, kind="ExternalInput")
    eidx = nc.dram_tensor("eidx", [128, TT * 8], I16, kind="ExternalInput")
    erij = nc.dram_tensor("erij", [128, TT * 3], F32, kind="ExternalInput")
    outT = nc.dram_tensor("outT", [128, LCP], F32, kind="ExternalOutput")
    mshard = nc.dram_tensor("mshard", [SHARD, 384], BF16)
    mrec = nc.dram_tensor("mrec", [NPAD, 384], BF16, addr_space="Shared")

    def sv(idx, col):
        c = idx * 6 + col
        return svec_sb[:, c:c + 1]

    def wslice(idx, layer):
        k = 3 * idx + layer
        return wsb[:, k * 128:(k + 1) * 128]

    with tile.TileContext(nc) as tc:
        from contextlib import ExitStack
        es = ExitStack()
        cst = es.enter_context(tc.tile_pool(name="cst", bufs=1))

        cst_sb = cst.tile([128, CCOLS], F32)
        nc.sync.dma_start(out=cst_sb[:], in_=cstd[:])
        o = 0
        wsb = cst_sb[:, o:o + 1920]; o += 1920
        svec_sb = cst_sb[:, o:o + 30]; o += 30
        crow_sb = cst_sb[:, o:o + 176]; o += 176
        grep_sb = cst_sb[:, o:o + 384]; o += 384
        pdm_sb = cst_sb[:, o:o + 512]; o += 512
        xtl_sb = cst_sb[:, o:o + LCP]; o += LCP
        cstb_sb = cst.tile([128, 128 + TT], BF16)
        nc.sync.dma_start(out=cstb_sb[:], in_=cstb[:])
        eidx_sb = cst.tile([128, TT * 8], I16)
        nc.sync.dma_start(out=eidx_sb[:], in_=eidx[:])
        ident = cst.tile([128, 128], F32)
        make_identity(nc, ident[:])
        identb = cst.tile([128, 128], BF16)
        nc.vector.tensor_copy(identb[:], ident[:])
        cterm = cst.tile([128, LCP], F32)
        inp_sb = cst.tile([128, LCP], F32)

        grep_b = cst.tile([128, 384], BF16)
        nc.vector.tensor_copy(grep_b[:], grep_sb)

        iotae, erele = cstb_sb[:, 0:128], cstb_sb[:, 128:128 + TT]
        gk_b = crow_sb[:, 128:144].rearrange("p (o k) -> p o k", o=1)
        k15_b = crow_sb[:, 144:160].rearrange("p (o k) -> p o k", o=1)
        lnb_b = crow_sb[:, 160:176].rearrange("p (o k) -> p o k", o=1)

        def resmlp_chunk(idx, x_ap, ncols, sbp, psp, out_ap, out_dve=False):
            sw = sbp.tile([128, ncols], F32, tag="sw", bufs=3, name="sw")
            nc.scalar.activation(sw[:], x_ap, AF.Silu, scale=sv(idx, 0))
            h1 = psp.tile([128, ncols], F32, space="PSUM", tag="h", bufs=6,
                          name="h1")
            nc.tensor.matmul(out=h1[:], lhsT=wslice(idx, 0), rhs=sw[:],
                             start=True, stop=True)
            sw2 = sbp.tile([128, ncols], F32, tag="sw2", bufs=3, name="sw2")
            nc.scalar.activation(sw2[:], h1[:], AF.Silu, scale=sv(idx, 1),
                                 bias=sv(idx, 3))
            h2 = psp.tile([128, ncols], F32, space="PSUM", tag="h", bufs=6,
                          name="h2")
            nc.tensor.matmul(out=h2[:], lhsT=wslice(idx, 1), rhs=sw2[:],
                             start=True, stop=True)
            r = sbp.tile([128, ncols], F32, tag="r", bufs=3, name="r")
            nc.vector.scalar_tensor_tensor(out=r[:], in0=h2[:], scalar=sv(idx, 4),
                                           in1=x_ap, op0=ALU.add, op1=ALU.add)
            sw3 = sbp.tile([128, ncols], F32, tag="sw3", bufs=3, name="sw3")
            nc.scalar.activation(sw3[:], r[:], AF.Silu, scale=sv(idx, 2))
            h3 = psp.tile([128, ncols], F32, space="PSUM", tag="h", bufs=6,
                          name="h3")
            nc.tensor.matmul(out=h3[:], lhsT=wslice(idx, 2), rhs=sw3[:],
                             start=True, stop=True)
            if out_dve:
                nc.vector.tensor_scalar(out=out_ap, in0=h3[:],
                                        scalar1=sv(idx, 5), scalar2=None,
                                        op0=ALU.add)
            else:
                nc.scalar.activation(out_ap, h3[:], AF.Identity,
                                     bias=sv(idx, 5))
            return h3

        # ------------------------------------------------------------------
        # node phase: mlp(1..3) over all nodes -> node-major records in HBM
        # ------------------------------------------------------------------
        stage_dmas = []
        with (
            tc.tile_pool(name="xt", bufs=1) as xtp,
            tc.tile_pool(name="nod", bufs=3) as nod,
            tc.tile_pool(name="stg", bufs=3) as stg,
            tc.tile_pool(name="nps", bufs=6, space="PSUM") as nps,
            tc.tile_pool(name="tps", bufs=2, space="PSUM") as tps,
        ):
            xt_sb = xtp.tile([128, SHARD], F32)
            nc.sync.dma_start(out=xt_sb[:], in_=xtT[:])
            for ch in range(NCHUNK):
                x_ap = xt_sb[:, ch * 512:(ch + 1) * 512]
                stage_t = stg.tile([128, 4, 384], BF16, tag="stage", name="stage")
                for idx in (1, 2, 3):
                    m_sb = nod.tile([128, 512], F32, tag="msb", name="msb")
                    resmlp_chunk(idx, x_ap, 512, nod, nps, m_sb[:],
                                 out_dve=True)
                    for b in range(4):
                        tp = tps.tile([128, 128], F32, space="PSUM", tag="tp",
                                      name="tp")
                        nc.tensor.transpose(out=tp[:],
                                            in_=m_sb[:, b * 128:(b + 1) * 128],
                                            identity=ident[:])
                        dst = stage_t[:, b, (idx - 1) * 128: idx * 128]
                        if b < 2:
                            nc.scalar.copy(dst, tp[:])
                        else:
                            nc.vector.tensor_copy(dst, tp[:])
                dst = mshard[ch * 512:(ch + 1) * 512, :].rearrange(
                    "(c p) f -> p c f", p=128)
                dma = nc.sync.dma_start(out=dst, in_=stage_t[:])
                stage_dmas.append(dma)
            for ch in range(LCP // 512):
                x_ap = xtl_sb[:, ch * 512:(ch + 1) * 512]
                resmlp_chunk(0, x_ap, 512, nod, nps,
                             cterm[:, ch * 512:(ch + 1) * 512])

        # all-gather the per-core record shards into the full record table
        cc = nc.gpsimd.collective_compute(
            "AllGather", ALU.bypass,
            replica_groups=[list(range(N_CORES))],
            ins=[mshard[:]], outs=[mrec[:]])
        for sd in stage_dmas:
            add_dep_helper(cc.ins, sd.ins, reason="shard before AG")

        # ------------------------------------------------------------------
        # batched radial pass: bern/u/Y2 for ALL edge tiles (bf16 outputs)
        # ------------------------------------------------------------------
        TPF = os.environ.get("KB_TPF", "0") == "1"   # fp32 bern/transpose path
        bern_all = cst.tile([128, TT, 32], F32 if TPF else BF16)
        ub_all = cst.tile([128, TT, 3], BF16)
        y2b_all = cst.tile([128, TT, 5], BF16)
        RC = 2                      # radial chunks
        rch = (TT + RC - 1) // RC
        with (
            tc.tile_pool(name="rij", bufs=2) as rijp,
            tc.tile_pool(name="rsc", bufs=2) as rsc,
        ):
            for c in range(RC):
                t0 = c * rch
                ct = min(rch, TT - t0)
                if ct <= 0:
                    continue
                rij = rijp.tile([128, rch, 3], F32, tag="rij", name="rij")
                nc.sync.dma_start(out=rij[:, 0:ct, :],
                                  in_=erij[:, t0 * 3:(t0 + ct) * 3]
                                  .rearrange("p (t c) -> p t c", c=3))
                rijv = rij[:, 0:ct, :]

                def sc1(tag):
                    t = rsc.tile([128, rch, 1], F32, tag=tag, name=tag)
                    return t[:, 0:ct, :]

                sq = rsc.tile([128, rch, 3], F32, tag="sq", name="sq")[:, 0:ct, :]
                nc.vector.tensor_tensor(out=sq, in0=rijv, in1=rijv, op=ALU.mult)
                d2 = sc1("d2")
                nc.vector.tensor_reduce(out=d2, in_=sq, axis=mybir.AxisListType.X,
                                        op=ALU.add)
                # d = exp(0.5*ln(d2+eps)); inv_d = exp(-0.5*ln(d2+eps))
                lgd = sc1("lgd")
                nc.scalar.activation(lgd, d2, AF.Ln, bias=1e-12)
                d = sc1("d")
                nc.scalar.activation(d, lgd, AF.Exp, scale=0.5)
                inv_d = sc1("ivd")
                nc.scalar.activation(inv_d, lgd, AF.Exp, scale=-0.5)
                u = rsc.tile([128, rch, 3], F32, tag="u", name="u")[:, 0:ct, :]
                nc.vector.tensor_tensor(out=u, in0=rijv,
                                        in1=inv_d.to_broadcast([128, ct, 3]),
                                        op=ALU.mult)
                usq = rsc.tile([128, rch, 3], F32, tag="usq", name="usq")[:, 0:ct, :]
                nc.vector.tensor_tensor(out=usq, in0=u, in1=u, op=ALU.mult)
                y2 = rsc.tile([128, rch, 5], F32, tag="y2", name="y2")[:, 0:ct, :]
                nc.vector.tensor_tensor(out=y2[:, :, 0:1], in0=u[:, :, 0:1],
                                        in1=u[:, :, 1:2], op=ALU.mult)
                nc.vector.tensor_tensor(out=y2[:, :, 1:2], in0=u[:, :, 0:1],
                                        in1=u[:, :, 2:3], op=ALU.mult)
                nc.vector.tensor_tensor(out=y2[:, :, 2:3], in0=u[:, :, 1:2],
                                        in1=u[:, :, 2:3], op=ALU.mult)
                nc.vector.tensor_tensor(out=y2[:, :, 3:4], in0=usq[:, :, 0:1],
                                        in1=usq[:, :, 1:2], op=ALU.subtract)
                nc.vector.tensor_scalar(out=y2[:, :, 4:5], in0=usq[:, :, 2:3],
                                        scalar1=3.0, scalar2=-1.0,
                                        op0=ALU.mult, op1=ALU.add)
                nc.vector.tensor_copy(ub_all[:, t0:t0 + ct, :], u)
                nc.vector.tensor_copy(y2b_all[:, t0:t0 + ct, :], y2)
                rho = sc1("rho")
                nc.scalar.activation(rho, d, AF.Exp, scale=-GAMMA)
                om = sc1("om")
                nc.vector.tensor_scalar(out=om, in0=rho, scalar1=-1.0,
                                        scalar2=1.0, op0=ALU.mult, op1=ALU.add)
                nc.vector.tensor_scalar(out=om, in0=om, scalar1=1e-38,
                                        scalar2=None, op0=ALU.max)
                lg = sc1("lg")
                nc.scalar.activation(lg, om, AF.Ln)
                den = sc1("den")
                nc.vector.tensor_scalar(out=den, in0=d2, scalar1=-1.0,
                                        scalar2=R_CUT * R_CUT,
                                        op0=ALU.mult, op1=ALU.add)
                rden = sc1("rdn")
                nc.vector.reciprocal(rden, den)
                mme = sc1("mme")
                nc.vector.tensor_tensor(out=mme, in0=d2, in1=rden, op=ALU.mult)
                msk = sc1("msk")
                nc.vector.tensor_scalar(out=msk, in0=d, scalar1=R_CUT,
                                        scalar2=None, op0=ALU.is_ge)
                arge = sc1("age")
                nc.vector.scalar_tensor_tensor(out=arge, in0=msk, scalar=1e30,
                                               in1=mme, op0=ALU.mult, op1=ALU.add)
                arg = rsc.tile([128, rch, 16], F32, tag="arg", name="arg")
                argv = arg[:, 0:ct, :]
                nc.vector.tensor_tensor(out=argv,
                                        in0=d.to_broadcast([128, ct, 16]),
                                        in1=gk_b.to_broadcast([128, ct, 16]),
                                        op=ALU.mult)
                t16 = rsc.tile([128, rch, 16], F32, tag="t16", name="t16")
                nc.vector.tensor_tensor(out=t16[:, 0:ct, :],
                                        in0=lg.to_broadcast([128, ct, 16]),
                                        in1=k15_b.to_broadcast([128, ct, 16]),
                                        op=ALU.mult)
                nc.vector.tensor_tensor(out=argv, in0=argv,
                                        in1=t16[:, 0:ct, :], op=ALU.add)
                nc.vector.tensor_tensor(out=argv, in0=argv,
                                        in1=arge.to_broadcast([128, ct, 16]),
                                        op=ALU.subtract)
                nc.vector.tensor_tensor(out=argv, in0=argv,
                                        in1=lnb_b.to_broadcast([128, ct, 16]),
                                        op=ALU.add)
                nc.vector.memset(bern_all[:, t0:t0 + ct, 16:32], 0.0)
                nc.scalar.activation(bern_all[:, t0:t0 + ct, 0:16], argv, AF.Exp)

        # ------------------------------------------------------------------
        # edge phase: per-window gather + scatter-matmul accumulate
        # ------------------------------------------------------------------
        with (
            tc.tile_pool(name="rec", bufs=2) as recp,
            tc.tile_pool(name="ohp", bufs=3) as ohp,
            tc.tile_pool(name="rt", bufs=3) as rtp,
            tc.tile_pool(name="ap_", bufs=3) as app,
            tc.tile_pool(name="epi", bufs=2) as epip,
            tc.tile_pool(name="acc_ps", bufs=1, space="PSUM") as accp,
            tc.tile_pool(name="rg_ps", bufs=1, space="PSUM") as rgp,
            tc.tile_pool(name="scr_ps", bufs=2, space="PSUM") as scrp,
            tc.tile_pool(name="tp_ps", bufs=1, space="PSUM") as tpp,
        ):
            for w in range(NW):
                nt = nt_list[w]
                T0 = int(starts[w])

                rec = recp.tile([128, ntmax, 384], BF16, tag="rec", name="rec")
                GB = int(os.environ.get("KB_GB", "0")) or nt
                for g0 in range(0, nt, GB):
                    gn = min(GB, nt - g0)
                    g = nc.gpsimd.dma_gather(
                        rec[:, g0:g0 + gn, :], mrec[:],
                        eidx_sb[:, (T0 + g0) * 8:(T0 + g0 + gn) * 8],
                        gn * 128, gn * 128, 384)
                    add_dep_helper(g.ins, cc.ins, reason="AG before gather")

                # separate PSUM banks: one accumulation group per bank
                acc_s = accp.tile([128, W], F32, space="PSUM", tag="acc_s",
                                  name="acc_s")[:]
                acc_p = accp.tile([128, 3 * W], F32, space="PSUM", tag="acc_p",
                                  name="acc_p")[:]
                acc_d = accp.tile([128, 5 * W], F32, space="PSUM", tag="acc_d",
                                  name="acc_d")[:]

                for gix in range((nt + 3) // 4):
                    gsz = min(4, nt - gix * 4)
                    tp = tpp.tile([128, 128], F32 if TPF else BF16,
                                  space="PSUM", tag="tscr", name="tscr")
                    nc.tensor.transpose(
                        out=tp[0:32 * gsz, 0:128],
                        in_=bern_all[:, T0 + gix * 4: T0 + gix * 4 + gsz, :],
                        identity=ident[:] if TPF else identb[:])
                    radT = rtp.tile([128, 128], BF16, tag="radT", name="radT")
                    nc.scalar.copy(radT[0:32 * gsz, :], tp[0:32 * gsz, 0:128])

                    # batched one-hot construction for the group (bf16)
                    ohb = ohp.tile([128, 4, 9, W], BF16, tag="oh", name="oh")
                    irelb = erele[:, T0 + gix * 4: T0 + gix * 4 + gsz] \
                        .rearrange("p (t o) -> p t o", o=1)
                    nc.vector.tensor_tensor(
                        out=ohb[:, 0:gsz, 0, :],
                        in0=iotae[:, 0:W].rearrange("p (o n) -> p o n", o=1)
                        .to_broadcast([128, gsz, W]),
                        in1=irelb.to_broadcast([128, gsz, W]),
                        op=ALU.is_equal)
                    ubg = ub_all[:, T0 + gix * 4:T0 + gix * 4 + gsz, :] \
                        .rearrange("p t (c o) -> p t c o", o=1)
                    nc.vector.tensor_tensor(
                        out=ohb[:, 0:gsz, 1:4, :],
                        in0=ohb[:, 0:gsz, 0:1, :].to_broadcast([128, gsz, 3, W]),
                        in1=ubg.to_broadcast([128, gsz, 3, W]),
                        op=ALU.mult)
                    y2g = y2b_all[:, T0 + gix * 4:T0 + gix * 4 + gsz, :] \
                        .rearrange("p t (c o) -> p t c o", o=1)
                    nc.vector.tensor_tensor(
                        out=ohb[:, 0:gsz, 4:9, :],
                        in0=ohb[:, 0:gsz, 0:1, :].to_broadcast([128, gsz, 5, W]),
                        in1=y2g.to_broadcast([128, gsz, 5, W]),
                        op=ALU.mult)

                    for pair in range((gsz + 1) // 2):
                        psz = min(2, gsz - pair * 2)
                        radG2 = rgp.tile([128, 2, 512], F32, space="PSUM",
                                         tag="rg", name="rg")
                        for k in range(psz):
                            q = pair * 2 + k
                            nc.tensor.matmul(
                                out=radG2[:, k, 0:384],
                                lhsT=radT[32 * q:32 * q + 32, :],
                                rhs=grep_b[32 * q:32 * q + 32, :],
                                start=True, stop=True,
                                tile_position=(32 * q, 0))
                        rgsb = app.tile([128, 2, 384], BF16, tag="rgsb",
                                        name="rgsb")
                        nc.scalar.copy(rgsb[:, 0:psz, :],
                                       radG2[:, 0:psz, 0:384])
                        A2 = app.tile([128, 2, 384], BF16, tag="A", name="A")
                        t0_ = gix * 4 + pair * 2
                        nc.vector.tensor_tensor(
                            out=A2[:, 0:psz, :], in0=rec[:, t0_:t0_ + psz, :],
                            in1=rgsb[:, 0:psz, :], op=ALU.mult)
                        for k in range(psz):
                            t = t0_ + k
                            st, sp = (t == 0), (t == nt - 1)
                            A_ = A2[:, k, :]
                            oh_ = ohb[:, t - gix * 4, :, :]
                            nc.tensor.matmul(out=acc_s, lhsT=A_[:, 0:128],
                                             rhs=oh_[:, 0, :], start=st, stop=sp)
                            nc.tensor.matmul(out=acc_p, lhsT=A_[:, 128:256],
                                             rhs=oh_[:, 1:4, :], start=st,
                                             stop=sp)
                            nc.tensor.matmul(out=acc_d, lhsT=A_[:, 256:384],
                                             rhs=oh_[:, 4:9, :], start=st,
                                             stop=sp)

                # ---- window epilogue: P/D invariants -> inp_sb column block
                qsb = epip.tile([128, 9 * W], F32, tag="qsb", name="qsb")
                nc.scalar.copy(qsb[:, W:4 * W], acc_p)
                nc.scalar.copy(qsb[:, 4 * W:9 * W], acc_d)
                inpw = inp_sb[:, w * W:(w + 1) * W]
                nc.vector.tensor_tensor(out=inpw, in0=acc_s,
                                        in1=cterm[:, w * W:(w + 1) * W],
                                        op=ALU.add)
                t1 = scrp.tile([128, 512], F32, space="PSUM", tag="scr", name="t1")
                t2 = scrp.tile([128, 512], F32, space="PSUM", tag="scr", name="t2")
                nc.tensor.matmul(out=t1[:, 0:3 * W], lhsT=pdm_sb[:, 0:128],
                                 rhs=qsb[:, W:4 * W], start=True, stop=True)
                nc.tensor.matmul(out=t2[:, 0:3 * W], lhsT=pdm_sb[:, 128:256],
                                 rhs=qsb[:, W:4 * W], start=True, stop=True)
                t1sb = epip.tile([128, 512], F32, tag="t1sb", name="t1sb")
                nc.scalar.copy(t1sb[:, 0:3 * W], t1[:, 0:3 * W])
                pp = epip.tile([128, W, 5], F32, tag="pp", name="pp")
                nc.vector.tensor_tensor(
                    out=pp[:, :, 0:3].rearrange("p n c -> p c n"),
                    in0=t1sb[:, 0:3 * W].rearrange("p (c n) -> p c n", n=W),
                    in1=t2[:, 0:3 * W].rearrange("p (c n) -> p c n", n=W),
                    op=ALU.mult)
                red = epip.tile([128, W], F32, tag="red", name="red")
                nc.vector.tensor_reduce(out=red[:], in_=pp[:, :, 0:3],
                                        axis=mybir.AxisListType.X, op=ALU.add)
                nc.vector.tensor_tensor(out=inpw, in0=inpw, in1=red[:],
                                        op=ALU.add)
                t1d = scrp.tile([128, 512], F32, space="PSUM", tag="scr", name="t1d")
                t2d = scrp.tile([128, 512], F32, space="PSUM", tag="scr", name="t2d")
                nc.tensor.matmul(out=t1d[:, 0:4 * W], lhsT=pdm_sb[:, 256:384],
                                 rhs=qsb[:, 4 * W:8 * W], start=True, stop=True)
                nc.tensor.matmul(out=t2d[:, 0:4 * W], lhsT=pdm_sb[:, 384:512],
                                 rhs=qsb[:, 4 * W:8 * W], start=True, stop=True)
                t1dsb = epip.tile([128, 512], F32, tag="t1sb", name="t1dsb")
                nc.scalar.copy(t1dsb[:, 0:4 * W], t1d[:, 0:4 * W])
                ppd = epip.tile([128, W, 5], F32, tag="pp", name="ppd")
                nc.vector.tensor_tensor(
                    out=ppd[:, :, 0:4].rearrange("p n c -> p c n"),
                    in0=t1dsb[:, 0:4 * W].rearrange("p (c n) -> p c n", n=W),
                    in1=t2d[:, 0:4 * W].rearrange("p (c n) -> p c n", n=W),
                    op=ALU.mult)
                t1e = scrp.tile([128, 512], F32, space="PSUM", tag="scr", name="t1e")
                t2e = scrp.tile([128, 512], F32, space="PSUM", tag="scr", name="t2e")
                nc.tensor.matmul(out=t1e[:, 0:W], lhsT=pdm_sb[:, 256:384],
                                 rhs=qsb[:, 8 * W:9 * W], start=True, stop=True)
                nc.tensor.matmul(out=t2e[:, 0:W], lhsT=pdm_sb[:, 384:512],
                                 rhs=qsb[:, 8 * W:9 * W], start=True, stop=True)
                t1esb = epip.tile([128, W], F32, tag="t1esb", name="t1esb")
                nc.scalar.copy(t1esb[:], t1e[:, 0:W])
                nc.vector.tensor_tensor(
                    out=ppd[:, :, 4:5].rearrange("p n c -> p c n"),
                    in0=t1esb[:].rearrange("p (c n) -> p c n", n=W),
                    in1=t2e[:, 0:W].rearrange("p (c n) -> p c n", n=W),
                    op=ALU.mult)
                redd = epip.tile([128, W], F32, tag="red", name="redd")
                nc.vector.tensor_reduce(out=redd[:], in_=ppd[:],
                                        axis=mybir.AxisListType.X, op=ALU.add)
                nc.vector.tensor_tensor(out=inpw, in0=inpw, in1=redd[:],
                                        op=ALU.add)

        # ---- batched final MLP over all local nodes ----
        with (
            tc.tile_pool(name="fin", bufs=3) as fin,
            tc.tile_pool(name="fps", bufs=6, space="PSUM") as fps,
        ):
            for ch in range(LCP // 512):
                outw = fin.tile([128, 512], F32, tag="outw", name="outw")
                resmlp_chunk(4, inp_sb[:, ch * 512:(ch + 1) * 512], 512,
                             fin, fps, outw[:])
                nc.sync.dma_start(out=outT[:, ch * 512:(ch + 1) * 512],
                                  in_=outw[:])
        es.close()
    nc.compile()
    return nc


# ----------------------------------------------------------------------------
# host side
# ----------------------------------------------------------------------------

def _prep_host(xyz, x_tilde, nbrs, W1, b1, W2, b2, W3, b3, alpha, beta,
               G_s, G_p, G_d, P_1, P_2, D_1, D_2):
    xyz = np.asarray(xyz, np.float32)
    x_tilde = np.asarray(x_tilde, np.float32)
    nbrs = np.asarray(nbrs)
    i = nbrs[:, 0].astype(np.int64)
    j = nbrs[:, 1].astype(np.int64)
    E = i.shape[0]

    r_ij = (xyz[j] - xyz[i]).astype(np.float32)

    core = i // NPC
    iloc = i - core * NPC
    w = iloc // W
    irel = (iloc % W).astype(np.float32)
    key = core * NW + w
    order = np.argsort(key, kind="stable")
    cnt = np.bincount(key, minlength=N_CORES * NW).reshape(N_CORES, NW)
    n_t = np.maximum(1, -(-cnt.max(axis=0) // 128)).astype(int)
    TT = int(n_t.sum())
    starts = np.concatenate([[0], np.cumsum(n_t)]).astype(int)
    EPAD = TT * 128

    j_pad = np.zeros((N_CORES, EPAD), np.int64)
    irel_pad = np.full((N_CORES, EPAD), 200.0, np.float32)
    rij_pad = np.zeros((N_CORES, EPAD, 3), np.float32)

    cnt_flat = cnt.reshape(-1)
    grp_start = np.concatenate([[0], np.cumsum(cnt_flat)])[:-1]
    pos_in_grp = np.arange(E) - np.repeat(grp_start, cnt_flat)
    core_s = core[order]
    w_s = w[order]
    slot = starts[w_s] * 128 + pos_in_grp
    j_pad[core_s, slot] = j[order]
    irel_pad[core_s, slot] = irel[order]
    rij_pad[core_s, slot] = r_ij[order]

    eidx = np.zeros((N_CORES, 128, TT * 8), np.int16)
    for wi in range(NW):
        nt = int(n_t[wi]); base = int(starts[wi])
        jw = j_pad[:, base * 128:(base + nt) * 128]
        c = np.arange(nt * 8)
        t, q = c // 8, c % 8
        r16 = np.arange(16)
        e_ix = t[None, :] * 128 + r16[:, None] + 16 * q[None, :]
        blk = jw[:, e_ix].astype(np.int16)
        eidx[:, :, base * 8:(base + nt) * 8] = np.tile(blk, (1, 8, 1))

    erel = irel_pad.reshape(N_CORES, TT, 128).transpose(0, 2, 1).copy()
    erij = rij_pad.reshape(N_CORES, TT, 128, 3).transpose(0, 2, 1, 3) \
        .reshape(N_CORES, 128, TT * 3).copy()

    alpha = np.asarray(alpha, np.float64)
    beta = np.asarray(beta, np.float64)
    W1 = np.asarray(W1, np.float64); W2 = np.asarray(W2, np.float64)
    W3 = np.asarray(W3, np.float64)
    b1 = np.asarray(b1, np.float64); b2 = np.asarray(b2, np.float64)
    b3 = np.asarray(b3, np.float64)
    assert np.all(np.abs(beta) > 1e-6), "beta==0 unsupported by silu fold"

    wmats, svcols = [], np.zeros((128, 30), np.float32)
    for idx in range(5):
        a, b = alpha[idx], beta[idx]
        wmats += [(a[0] / b[0])[:, None] * W1[idx],
                  (a[1] / b[1])[:, None] * W2[idx],
                  (a[2] / b[2])[:, None] * W3[idx]]
        svcols[:, idx * 6 + 0] = b[0]
        svcols[:, idx * 6 + 1] = b[1]
        svcols[:, idx * 6 + 2] = b[2]
        svcols[:, idx * 6 + 3] = b[1] * b1[idx]
        svcols[:, idx * 6 + 4] = b2[idx]
        svcols[:, idx * 6 + 5] = b3[idx]
    wst = np.stack(wmats).astype(np.float32).transpose(1, 0, 2) \
        .reshape(128, 15 * 128).copy()

    crow = np.zeros((128, 176), np.float32)
    crow[:, 0:128] = np.arange(128, dtype=np.float32)[None, :]
    ks = np.arange(16, dtype=np.float64)
    crow[:, 128:144] = (-GAMMA * ks)[None, :]
    crow[:, 144:160] = (15.0 - ks)[None, :]
    crow[:, 160:176] = np.log(_BINOM)[None, :]

    grep_np = np.zeros((128, 384), np.float32)
    for q in range(4):
        for X, G in enumerate([G_s, G_p, G_d]):
            grep_np[32 * q:32 * q + 16, X * 128:(X + 1) * 128] = \
                np.asarray(G, np.float32).T
    pdm_np = np.concatenate([np.asarray(M, np.float32).T for M in
                             (P_1, P_2, D_1, D_2)], axis=1).copy()

    xtT_np = np.zeros((128, NPAD), np.float32)
    xtT_np[:, :N_NODES] = x_tilde.T
    xtl_np = np.zeros((N_CORES, 128, LOCPAD), np.float32)
    for cix in range(N_CORES):
        xtl_np[cix, :, :NPC] = x_tilde[cix * NPC:(cix + 1) * NPC].T

    import ml_dtypes
    iota_bf = np.arange(128, dtype=np.float32)[None, :].repeat(128, 0)
    in_maps = []
    for cix in range(N_CORES):
        cstd = np.concatenate(
            [wst, svcols, crow, grep_np, pdm_np, xtl_np[cix], erel[cix]],
            axis=1).astype(np.float32)
        cstb = np.concatenate([iota_bf, erel[cix]], axis=1) \
            .astype(ml_dtypes.bfloat16)
        in_maps.append({
            "xtT": xtT_np[:, cix * SHARD:(cix + 1) * SHARD].copy(),
            "cstd": cstd, "cstb": cstb,
            "eidx": eidx[cix], "erij": erij[cix],
        })
    return tuple(int(x) for x in n_t), in_maps


def kernel(**inputs) -> np.ndarray:
    global LAST_EXEC_NS
    n_t, in_maps = _prep_host(**inputs)
    key = n_t + (os.environ.get("KB_GB", "0"), os.environ.get("KB_TPF", "0"))
    if key not in _prog_cache:
        _prog_cache[key] = _build_program(n_t)
    nc = _prog_cache[key]

    trace = os.environ.get("KBENCH_TRACE", "0") == "1"
    res = run_bass_kernel_spmd(nc, in_maps, core_ids=list(range(N_CORES)),
                               trace=trace)
    if trace:
        LAST_EXEC_NS = res.exec_time_ns
        global LAST_RESULT
        LAST_RESULT = res
        if res.instructions_and_trace:
            print(f"trace path: {res.instructions_and_trace[1]}")

    out = np.empty((N_NODES, FEAT), np.float32)
    for cix in range(N_CORES):
        out[cix * NPC:(cix + 1) * NPC] = res.results[cix]["outT"][:, :NPC].T
    return out


# revision 27
# speedup vs baseline: 2.5951x; 1.0645x over previous
"""Trainium2 Bass kernel for nn_LocalInteraction (SpookyNet-style local interaction).

Strategy (8 NeuronCores, SPMD):
  - Edges sharded by DESTINATION node: core c owns nodes [2000c, 2000c+2000)
    plus all edges whose receiver i lies there; NW windows of W dest nodes,
    padded to a uniform per-window tile grid shared by all cores (one NEFF).
  - Node phase (replicated, feature-major): the three edge MLPs are computed
    per node (mlp(x_j) == mlp(x_tilde)[j]) over all 16000 nodes, transposed to
    node-major records [node, 384] in HBM.
  - Radial basis computed in ONE batched pass over all edge tiles (exp/log
    formulation, single ACT table set) -> bern/u/Y2 in bf16.
  - Edge phase: records gathered by j via one dma_gather per window
    (edge-major); scaled one-hots built as batched broadcast tensor_tensor
    ops (bf16); segment-sum is 4 PSUM-accumulated matmuls per tile.
  - Window epilogue: P/D invariant contractions accumulate into an SBUF
    inp buffer; the final MLP runs ONCE batched over all local nodes.
"""
import sys, os, math
if not any("trn_rl_repo" in p or "simrepo" in p for p in sys.path):
    sys.path.insert(0, "/opt/trn_rl_repo")
import numpy as np

import concourse.bass as bass
import concourse.bacc as bacc
import concourse.mybir as mybir
import concourse.tile as tile
from concourse.bass_utils import run_bass_kernel_spmd
from concourse.masks import make_identity
from concourse.tile import add_dep_helper

F32 = mybir.dt.float32
BF16 = mybir.dt.bfloat16
I16 = mybir.dt.int16
AF = mybir.ActivationFunctionType
ALU = mybir.AluOpType

N_NODES = 16000
FEAT = 128
GAMMA = 0.5
R_CUT = 5.0
N_CORES = 8
NPC = N_NODES // N_CORES          # 2000 nodes per core
W = int(os.environ.get("KB_W", "64"))   # dest-window width
NW = (NPC + W - 1) // W           # windows per core
NPAD = 16384
SHARD = NPAD // N_CORES           # 2048 record-nodes per core
NCHUNK = SHARD // 512             # 4 chunks per core (sharded node phase)
LOCPAD = NW * W                   # padded local node count
_BINOM = np.array([math.comb(15, k) for k in range(16)], np.float64)

LAST_EXEC_NS = None
LAST_RESULT = None
_prog_cache = {}


def _build_program(n_t):
    nt_list = [x for x in n_t if not isinstance(x, str)]
    TT = sum(nt_list)
    starts = np.concatenate([[0], np.cumsum(nt_list)]).astype(int)
    ntmax = max(nt_list)

    nc = bacc.Bacc("TRN2", target_bir_lowering=False, debug=False,
                   num_devices=N_CORES)
    for v in (1e-12,):
        t_ = nc.alloc_sbuf_tensor(f"const-float32-{v}", [128, 1], F32)
        nc.gpsimd.memset(t_.ap(), v)
        nc.const_aps.aps[(F32, v)] = t_.ap()
    nc.all_engine_barrier()

    LCP = LOCPAD  # padded local columns (inp/cterm/out)
    CCOLS = 1920 + 30 + 176 + 384 + 512 + LCP + TT
    xtT = nc.dram_tensor("xtT", [128, SHARD], F32, kind="ExternalInput")
    cstd = nc.dram_tensor("cstd", [128, CCOLS], F32, kind="ExternalInput")
    cstb = nc.dram_tensor("cstb", [128, 128 + TT], BF16, kind="ExternalInput")
    eidx = nc.dram_tensor("eidx", [128, TT * 8], I16, kind="ExternalInput")
    erij = nc.dram_tensor("erij", [128, TT * 3], F32, kind="ExternalInput")
    outT = nc.dram_tensor("outT", [128, LCP], F32, kind="ExternalOutput")
    mshard = nc.dram_tensor("mshard", [SHARD, 384], BF16)
    mrec = nc.dram_tensor("mrec", [NPAD, 384], BF16, addr_space="Shared")

    def sv(idx, col):
        c = idx * 6 + col
        return svec_sb[:, c:c + 1]

    def wslice(idx, layer):
        k = 3 * idx + layer
        return wsb[:, k * 128:(k + 1) * 128]

    with tile.TileContext(nc) as tc:
        from contextlib import ExitStack
        es = ExitStack()
        cst = es.enter_context(tc.tile_pool(name="cst", bufs=1))

        cst_sb = cst.tile([128, CCOLS], F32)
        nc.sync.dma_start(out=cst_sb[:], in_=cstd[:])
        o = 0
        wsb = cst_sb[:, o:o + 1920]; o += 1920
        svec_sb = cst_sb[:, o:o + 30]; o += 30
        crow_sb = cst_sb[:, o:o + 176]; o += 176
        grep_sb = cst_sb[:, o:o + 384]; o += 384
        pdm_sb = cst_sb[:, o:o + 512]; o += 512
        xtl_sb = cst_sb[:, o:o + LCP]; o += LCP
        cstb_sb = cst.tile([128, 128 + TT], BF16)
        nc.sync.dma_start(out=cstb_sb[:], in_=cstb[:])
        eidx_sb = cst.tile([128, TT * 8], I16)
        nc.sync.dma_start(out=eidx_sb[:], in_=eidx[:])
        ident = cst.tile([128, 128], F32)
        make_identity(nc, ident[:])
        identb = cst.tile([128, 128], BF16)
        nc.vector.tensor_copy(identb[:], ident[:])
        cterm = cst.tile([128, LCP], F32)
        inp_sb = cst.tile([128, LCP], F32)

        grep_b = cst.tile([128, 384], BF16)
        nc.vector.tensor_copy(grep_b[:], grep_sb)

        iotae, erele = cstb_sb[:, 0:128], cstb_sb[:, 128:128 + TT]
        gk_b = crow_sb[:, 128:144].rearrange("p (o k) -> p o k", o=1)
        k15_b = crow_sb[:, 144:160].rearrange("p (o k) -> p o k", o=1)
        lnb_b = crow_sb[:, 160:176].rearrange("p (o k) -> p o k", o=1)

        def resmlp_chunk(idx, x_ap, ncols, sbp, psp, out_ap, out_dve=False,
                         ps_tag="h", ps_bufs=6):
            sw = sbp.tile([128, ncols], F32, tag="sw", bufs=3, name="sw")
            nc.scalar.activation(sw[:], x_ap, AF.Silu, scale=sv(idx, 0))
            h1 = psp.tile([128, ncols], F32, space="PSUM", tag=ps_tag,
                          bufs=ps_bufs, name="h1")
            nc.tensor.matmul(out=h1[:], lhsT=wslice(idx, 0), rhs=sw[:],
                             start=True, stop=True)
            sw2 = sbp.tile([128, ncols], F32, tag="sw2", bufs=3, name="sw2")
            nc.scalar.activation(sw2[:], h1[:], AF.Silu, scale=sv(idx, 1),
                                 bias=sv(idx, 3))
            h2 = psp.tile([128, ncols], F32, space="PSUM", tag=ps_tag,
                          bufs=ps_bufs, name="h2")
            nc.tensor.matmul(out=h2[:], lhsT=wslice(idx, 1), rhs=sw2[:],
                             start=True, stop=True)
            r = sbp.tile([128, ncols], F32, tag="r", bufs=3, name="r")
            nc.vector.scalar_tensor_tensor(out=r[:], in0=h2[:], scalar=sv(idx, 4),
                                           in1=x_ap, op0=ALU.add, op1=ALU.add)
            sw3 = sbp.tile([128, ncols], F32, tag="sw3", bufs=3, name="sw3")
            nc.scalar.activation(sw3[:], r[:], AF.Silu, scale=sv(idx, 2))
            h3 = psp.tile([128, ncols], F32, space="PSUM", tag=ps_tag,
                          bufs=ps_bufs, name="h3")
            nc.tensor.matmul(out=h3[:], lhsT=wslice(idx, 2), rhs=sw3[:],
                             start=True, stop=True)
            if out_dve:
                nc.vector.tensor_scalar(out=out_ap, in0=h3[:],
                                        scalar1=sv(idx, 5), scalar2=None,
                                        op0=ALU.add)
            else:
                nc.scalar.activation(out_ap, h3[:], AF.Identity,
                                     bias=sv(idx, 5))
            return h3

        # ------------------------------------------------------------------
        # node phase: mlp(1..3) over all nodes -> node-major records in HBM
        # ------------------------------------------------------------------
        stage_dmas = []
        cc_insts = []
        with (
            tc.tile_pool(name="xt", bufs=1) as xtp,
            tc.tile_pool(name="nod", bufs=3) as nod,
            tc.tile_pool(name="stg", bufs=3) as stg,
            tc.tile_pool(name="nps", bufs=6, space="PSUM") as nps,
            tc.tile_pool(name="tps", bufs=2, space="PSUM") as tps,
        ):
            xt_sb = xtp.tile([128, SHARD], F32)
            nc.sync.dma_start(out=xt_sb[:], in_=xtT[:])
            for ch in range(NCHUNK):
                x_ap = xt_sb[:, ch * 512:(ch + 1) * 512]
                stage_t = stg.tile([128, 4, 384], BF16, tag="stage", name="stage")
                for idx in (1, 2, 3):
                    m_sb = nod.tile([128, 512], F32, tag="msb", name="msb")
                    resmlp_chunk(idx, x_ap, 512, nod, nps, m_sb[:],
                                 out_dve=True)
                    for b in range(4):
                        tp = tps.tile([128, 128], F32, space="PSUM", tag="tp",
                                      name="tp")
                        nc.tensor.transpose(out=tp[:],
                                            in_=m_sb[:, b * 128:(b + 1) * 128],
                                            identity=ident[:])
                        dst = stage_t[:, b, (idx - 1) * 128: idx * 128]
                        if b < 2:
                            nc.scalar.copy(dst, tp[:])
                        else:
                            nc.vector.tensor_copy(dst, tp[:])
                dst = mshard[ch * 512:(ch + 1) * 512, :].rearrange(
                    "(c p) f -> p c f", p=128)
                dma = nc.sync.dma_start(out=dst, in_=stage_t[:])
                stage_dmas.append(dma)
                # all-gather this chunk right away (overlaps later chunks).
                # AG output: rank r's 512 rows land at mrec[4096*ch + 512*r];
                # the host bakes this permutation into the gather indices.
                cc = nc.gpsimd.collective_compute(
                    "AllGather", ALU.bypass,
                    replica_groups=[list(range(N_CORES))],
                    ins=[mshard[ch * 512:(ch + 1) * 512, :]],
                    outs=[mrec[ch * 8 * 512:(ch + 1) * 8 * 512, :]])
                add_dep_helper(cc.ins, dma.ins, reason="shard chunk before AG")
                cc_insts.append(cc)
            for ch in range(LCP // 512):
                x_ap = xtl_sb[:, ch * 512:(ch + 1) * 512]
                resmlp_chunk(0, x_ap, 512, nod, nps,
                             cterm[:, ch * 512:(ch + 1) * 512])

        # ------------------------------------------------------------------
        # batched radial pass: bern/u/Y2 for ALL edge tiles (bf16 outputs)
        # ------------------------------------------------------------------
        TPF = os.environ.get("KB_TPF", "0") == "1"   # fp32 bern/transpose path
        bern_all = cst.tile([128, TT, 32], F32 if TPF else BF16)
        ub_all = cst.tile([128, TT, 3], BF16)
        y2b_all = cst.tile([128, TT, 5], BF16)
        RC = 2                      # radial chunks
        rch = (TT + RC - 1) // RC
        with (
            tc.tile_pool(name="rij", bufs=2) as rijp,
            tc.tile_pool(name="rsc", bufs=2) as rsc,
        ):
            for c in range(RC):
                t0 = c * rch
                ct = min(rch, TT - t0)
                if ct <= 0:
                    continue
                rij = rijp.tile([128, rch, 3], F32, tag="rij", name="rij")
                nc.sync.dma_start(out=rij[:, 0:ct, :],
                                  in_=erij[:, t0 * 3:(t0 + ct) * 3]
                                  .rearrange("p (t c) -> p t c", c=3))
                rijv = rij[:, 0:ct, :]

                def sc1(tag):
                    t = rsc.tile([128, rch, 1], F32, tag=tag, name=tag)
                    return t[:, 0:ct, :]

                sq = rsc.tile([128, rch, 3], F32, tag="sq", name="sq")[:, 0:ct, :]
                nc.vector.tensor_tensor(out=sq, in0=rijv, in1=rijv, op=ALU.mult)
                d2 = sc1("d2")
                nc.vector.tensor_reduce(out=d2, in_=sq, axis=mybir.AxisListType.X,
                                        op=ALU.add)
                # d = exp(0.5*ln(d2+eps)); inv_d = exp(-0.5*ln(d2+eps))
                lgd = sc1("lgd")
                nc.scalar.activation(lgd, d2, AF.Ln, bias=1e-12)
                d = sc1("d")
                nc.scalar.activation(d, lgd, AF.Exp, scale=0.5)
                inv_d = sc1("ivd")
                nc.scalar.activation(inv_d, lgd, AF.Exp, scale=-0.5)
                u = rsc.tile([128, rch, 3], F32, tag="u", name="u")[:, 0:ct, :]
                nc.vector.tensor_tensor(out=u, in0=rijv,
                                        in1=inv_d.to_broadcast([128, ct, 3]),
                                        op=ALU.mult)
                usq = rsc.tile([128, rch, 3], F32, tag="usq", name="usq")[:, 0:ct, :]
                nc.vector.tensor_tensor(out=usq, in0=u, in1=u, op=ALU.mult)
                y2 = rsc.tile([128, rch, 5], F32, tag="y2", name="y2")[:, 0:ct, :]
                nc.vector.tensor_tensor(out=y2[:, :, 0:1], in0=u[:, :, 0:1],
                                        in1=u[:, :, 1:2], op=ALU.mult)
                nc.vector.tensor_tensor(out=y2[:, :, 1:2], in0=u[:, :, 0:1],
                                        in1=u[:, :, 2:3], op=ALU.mult)
                nc.vector.tensor_tensor(out=y2[:, :, 2:3], in0=u[:, :, 1:2],
                                        in1=u[:, :, 2:3], op=ALU.mult)
                nc.vector.tensor_tensor(out=y2[:, :, 3:4], in0=usq[:, :, 0:1],
                                        in1=usq[:, :, 1:2], op=ALU.subtract)
                nc.vector.tensor_scalar(out=y2[:, :, 4:5], in0=usq[:, :, 2:3],
                                        scalar1=3.0, scalar2=-1.0,
                                        op0=ALU.mult, op1=ALU.add)
                nc.vector.tensor_copy(ub_all[:, t0:t0 + ct, :], u)
                nc.vector.tensor_copy(y2b_all[:, t0:t0 + ct, :], y2)
                rho = sc1("rho")
                nc.scalar.activation(rho, d, AF.Exp, scale=-GAMMA)
                om = sc1("om")
                nc.vector.tensor_scalar(out=om, in0=rho, scalar1=-1.0,
                                        scalar2=1.0, op0=ALU.mult, op1=ALU.add)
                nc.vector.tensor_scalar(out=om, in0=om, scalar1=1e-38,
                                        scalar2=None, op0=ALU.max)
                lg = sc1("lg")
                nc.scalar.activation(lg, om, AF.Ln)
                den = sc1("den")
                nc.vector.tensor_scalar(out=den, in0=d2, scalar1=-1.0,
                                        scalar2=R_CUT * R_CUT,
                                        op0=ALU.mult, op1=ALU.add)
                rden = sc1("rdn")
                nc.vector.reciprocal(rden, den)
                mme = sc1("mme")
                nc.vector.tensor_tensor(out=mme, in0=d2, in1=rden, op=ALU.mult)
                msk = sc1("msk")
                nc.vector.tensor_scalar(out=msk, in0=d, scalar1=R_CUT,
                                        scalar2=None, op0=ALU.is_ge)
                arge = sc1("age")
                nc.vector.scalar_tensor_tensor(out=arge, in0=msk, scalar=1e30,
                                               in1=mme, op0=ALU.mult, op1=ALU.add)
                arg = rsc.tile([128, rch, 16], F32, tag="arg", name="arg")
                argv = arg[:, 0:ct, :]
                nc.vector.tensor_tensor(out=argv,
                                        in0=d.to_broadcast([128, ct, 16]),
                                        in1=gk_b.to_broadcast([128, ct, 16]),
                                        op=ALU.mult)
                t16 = rsc.tile([128, rch, 16], F32, tag="t16", name="t16")
                nc.vector.tensor_tensor(out=t16[:, 0:ct, :],
                                        in0=lg.to_broadcast([128, ct, 16]),
                                        in1=k15_b.to_broadcast([128, ct, 16]),
                                        op=ALU.mult)
                nc.vector.tensor_tensor(out=argv, in0=argv,
                                        in1=t16[:, 0:ct, :], op=ALU.add)
                nc.vector.tensor_tensor(out=argv, in0=argv,
                                        in1=arge.to_broadcast([128, ct, 16]),
                                        op=ALU.subtract)
                nc.vector.tensor_tensor(out=argv, in0=argv,
                                        in1=lnb_b.to_broadcast([128, ct, 16]),
                                        op=ALU.add)
                nc.vector.memset(bern_all[:, t0:t0 + ct, 16:32], 0.0)
                nc.scalar.activation(bern_all[:, t0:t0 + ct, 0:16], argv, AF.Exp)

        # ------------------------------------------------------------------
        # edge phase: per-window gather + scatter-matmul accumulate
        # ------------------------------------------------------------------
        with (
            tc.tile_pool(name="rec", bufs=3) as recp,
            tc.tile_pool(name="ohp", bufs=6) as ohp,
            tc.tile_pool(name="rt", bufs=6) as rtp,
            tc.tile_pool(name="ap_", bufs=5) as app,
            tc.tile_pool(name="epi", bufs=2) as epip,
            tc.tile_pool(name="acc_ps", bufs=1, space="PSUM") as accp,
            tc.tile_pool(name="rg_ps", bufs=1, space="PSUM") as rgp,
            tc.tile_pool(name="scr_ps", bufs=2, space="PSUM") as scrp,
            tc.tile_pool(name="tp_ps", bufs=1, space="PSUM") as tpp,
        ):
            for w in range(NW):
                nt = nt_list[w]
                T0 = int(starts[w])

                rec = recp.tile([128, ntmax, 384], BF16, tag="rec", name="rec")
                GB = int(os.environ.get("KB_GB", "0")) or nt
                for g0 in range(0, nt, GB):
                    gn = min(GB, nt - g0)
                    g = nc.gpsimd.dma_gather(
                        rec[:, g0:g0 + gn, :], mrec[:],
                        eidx_sb[:, (T0 + g0) * 8:(T0 + g0 + gn) * 8],
                        gn * 128, gn * 128, 384)
                    for cc in cc_insts:
                        add_dep_helper(g.ins, cc.ins, reason="AG before gather")

                # separate PSUM banks: one accumulation group per bank
                acc_s = accp.tile([128, W], F32, space="PSUM", tag="acc_s",
                                  name="acc_s")[:]
                acc_p = accp.tile([128, 3 * W], F32, space="PSUM", tag="acc_p",
                                  name="acc_p")[:]
                acc_d = accp.tile([128, 5 * W], F32, space="PSUM", tag="acc_d",
                                  name="acc_d")[:]

                for gix in range((nt + 3) // 4):
                    gsz = min(4, nt - gix * 4)
                    tp = tpp.tile([128, 128], F32 if TPF else BF16,
                                  space="PSUM", tag="tscr", name="tscr")
                    nc.tensor.transpose(
                        out=tp[0:32 * gsz, 0:128],
                        in_=bern_all[:, T0 + gix * 4: T0 + gix * 4 + gsz, :],
                        identity=ident[:] if TPF else identb[:])
                    radT = rtp.tile([128, 128], BF16, tag="radT", name="radT")
                    nc.scalar.copy(radT[0:32 * gsz, :], tp[0:32 * gsz, 0:128])

                    # batched one-hot construction for the group (bf16)
                    ohb = ohp.tile([128, 4, 9, W], BF16, tag="oh", name="oh")
                    irelb = erele[:, T0 + gix * 4: T0 + gix * 4 + gsz] \
                        .rearrange("p (t o) -> p t o", o=1)
                    nc.vector.tensor_tensor(
                        out=ohb[:, 0:gsz, 0, :],
                        in0=iotae[:, 0:W].rearrange("p (o n) -> p o n", o=1)
                        .to_broadcast([128, gsz, W]),
                        in1=irelb.to_broadcast([128, gsz, W]),
                        op=ALU.is_equal)
                    ubg = ub_all[:, T0 + gix * 4:T0 + gix * 4 + gsz, :] \
                        .rearrange("p t (c o) -> p t c o", o=1)
                    nc.vector.tensor_tensor(
                        out=ohb[:, 0:gsz, 1:4, :],
                        in0=ohb[:, 0:gsz, 0:1, :].to_broadcast([128, gsz, 3, W]),
                        in1=ubg.to_broadcast([128, gsz, 3, W]),
                        op=ALU.mult)
                    y2g = y2b_all[:, T0 + gix * 4:T0 + gix * 4 + gsz, :] \
                        .rearrange("p t (c o) -> p t c o", o=1)
                    nc.vector.tensor_tensor(
                        out=ohb[:, 0:gsz, 4:9, :],
                        in0=ohb[:, 0:gsz, 0:1, :].to_broadcast([128, gsz, 5, W]),
                        in1=y2g.to_broadcast([128, gsz, 5, W]),
                        op=ALU.mult)

                    for pair in range((gsz + 1) // 2):
                        psz = min(2, gsz - pair * 2)
                        radG2 = rgp.tile([128, 2, 512], F32, space="PSUM",
                                         tag="rg", name="rg")
                        for k in range(psz):
                            q = pair * 2 + k
                            nc.tensor.matmul(
                                out=radG2[:, k, 0:384],
                                lhsT=radT[32 * q:32 * q + 32, :],
                                rhs=grep_b[32 * q:32 * q + 32, :],
                                start=True, stop=True,
                                tile_position=(32 * q, 0))
                        rgsb = app.tile([128, 2, 384], BF16, tag="rgsb",
                                        name="rgsb")
                        nc.scalar.copy(rgsb[:, 0:psz, :],
                                       radG2[:, 0:psz, 0:384])
                        A2 = app.tile([128, 2, 384], BF16, tag="A", name="A")
                        t0_ = gix * 4 + pair * 2
                        nc.vector.tensor_tensor(
                            out=A2[:, 0:psz, :], in0=rec[:, t0_:t0_ + psz, :],
                            in1=rgsb[:, 0:psz, :], op=ALU.mult)
                        for k in range(psz):
                            t = t0_ + k
                            st, sp = (t == 0), (t == nt - 1)
                            A_ = A2[:, k, :]
                            oh_ = ohb[:, t - gix * 4, :, :]
                            nc.tensor.matmul(out=acc_s, lhsT=A_[:, 0:128],
                                             rhs=oh_[:, 0, :], start=st, stop=sp)
                            nc.tensor.matmul(out=acc_p, lhsT=A_[:, 128:256],
                                             rhs=oh_[:, 1:4, :], start=st,
                                             stop=sp)
                            nc.tensor.matmul(out=acc_d, lhsT=A_[:, 256:384],
                                             rhs=oh_[:, 4:9, :], start=st,
                                             stop=sp)

                # ---- window epilogue: P/D invariants -> inp_sb column block
                qsb = epip.tile([128, 9 * W], F32, tag="qsb", name="qsb")
                nc.scalar.copy(qsb[:, W:4 * W], acc_p)
                nc.scalar.copy(qsb[:, 4 * W:9 * W], acc_d)
                inpw = inp_sb[:, w * W:(w + 1) * W]
                nc.vector.tensor_tensor(out=inpw, in0=acc_s,
                                        in1=cterm[:, w * W:(w + 1) * W],
                                        op=ALU.add)
                t1 = scrp.tile([128, 512], F32, space="PSUM", tag="scr", name="t1")
                t2 = scrp.tile([128, 512], F32, space="PSUM", tag="scr", name="t2")
                nc.tensor.matmul(out=t1[:, 0:3 * W], lhsT=pdm_sb[:, 0:128],
                                 rhs=qsb[:, W:4 * W], start=True, stop=True)
                nc.tensor.matmul(out=t2[:, 0:3 * W], lhsT=pdm_sb[:, 128:256],
                                 rhs=qsb[:, W:4 * W], start=True, stop=True)
                t1sb = epip.tile([128, 512], F32, tag="t1sb", name="t1sb")
                nc.scalar.copy(t1sb[:, 0:3 * W], t1[:, 0:3 * W])
                pp = epip.tile([128, W, 5], F32, tag="pp", name="pp")
                nc.vector.tensor_tensor(
                    out=pp[:, :, 0:3].rearrange("p n c -> p c n"),
                    in0=t1sb[:, 0:3 * W].rearrange("p (c n) -> p c n", n=W),
                    in1=t2[:, 0:3 * W].rearrange("p (c n) -> p c n", n=W),
                    op=ALU.mult)
                red = epip.tile([128, W], F32, tag="red", name="red")
                nc.vector.tensor_reduce(out=red[:], in_=pp[:, :, 0:3],
                                        axis=mybir.AxisListType.X, op=ALU.add)
                nc.vector.tensor_tensor(out=inpw, in0=inpw, in1=red[:],
                                        op=ALU.add)
                t1d = scrp.tile([128, 512], F32, space="PSUM", tag="scr", name="t1d")
                t2d = scrp.tile([128, 512], F32, space="PSUM", tag="scr", name="t2d")
                nc.tensor.matmul(out=t1d[:, 0:4 * W], lhsT=pdm_sb[:, 256:384],
                                 rhs=qsb[:, 4 * W:8 * W], start=True, stop=True)
                nc.tensor.matmul(out=t2d[:, 0:4 * W], lhsT=pdm_sb[:, 384:512],
                                 rhs=qsb[:, 4 * W:8 * W], start=True, stop=True)
                t1dsb = epip.tile([128, 512], F32, tag="t1sb", name="t1dsb")
                nc.scalar.copy(t1dsb[:, 0:4 * W], t1d[:, 0:4 * W])
                ppd = epip.tile([128, W, 5], F32, tag="pp", name="ppd")
                nc.vector.tensor_tensor(
                    out=ppd[:, :, 0:4].rearrange("p n c -> p c n"),
                    in0=t1dsb[:, 0:4 * W].rearrange("p (c n) -> p c n", n=W),
                    in1=t2d[:, 0:4 * W].rearrange("p (c n) -> p c n", n=W),
                    op=ALU.mult)
                t1e = scrp.tile([128, 512], F32, space="PSUM", tag="scr", name="t1e")
                t2e = scrp.tile([128, 512], F32, space="PSUM", tag="scr", name="t2e")
                nc.tensor.matmul(out=t1e[:, 0:W], lhsT=pdm_sb[:, 256:384],
                                 rhs=qsb[:, 8 * W:9 * W], start=True, stop=True)
                nc.tensor.matmul(out=t2e[:, 0:W], lhsT=pdm_sb[:, 384:512],
                                 rhs=qsb[:, 8 * W:9 * W], start=True, stop=True)
                t1esb = epip.tile([128, W], F32, tag="t1esb", name="t1esb")
                nc.scalar.copy(t1esb[:], t1e[:, 0:W])
                nc.vector.tensor_tensor(
                    out=ppd[:, :, 4:5].rearrange("p n c -> p c n"),
                    in0=t1esb[:].rearrange("p (c n) -> p c n", n=W),
                    in1=t2e[:, 0:W].rearrange("p (c n) -> p c n", n=W),
                    op=ALU.mult)
                redd = epip.tile([128, W], F32, tag="red", name="redd")
                nc.vector.tensor_reduce(out=redd[:], in_=ppd[:],
                                        axis=mybir.AxisListType.X, op=ALU.add)
                nc.vector.tensor_tensor(out=inpw, in0=inpw, in1=redd[:],
                                        op=ALU.add)

                # final MLP for a 512-col block as soon as its windows done
                WPB = 512 // W
                if (w + 1) % WPB == 0:
                    ch = (w + 1) // WPB - 1
                    outw = epip.tile([128, 512], F32, tag="outw", name="outw")
                    resmlp_chunk(4, inp_sb[:, ch * 512:(ch + 1) * 512], 512,
                                 epip, scrp, outw[:], ps_tag="scr", ps_bufs=2)
                    nc.sync.dma_start(out=outT[:, ch * 512:(ch + 1) * 512],
                                      in_=outw[:])
        es.close()
    nc.compile()
    return nc


# ----------------------------------------------------------------------------
# host side
# ----------------------------------------------------------------------------

def _prep_host(xyz, x_tilde, nbrs, W1, b1, W2, b2, W3, b3, alpha, beta,
               G_s, G_p, G_d, P_1, P_2, D_1, D_2):
    xyz = np.asarray(xyz, np.float32)
    x_tilde = np.asarray(x_tilde, np.float32)
    nbrs = np.asarray(nbrs)
    i = nbrs[:, 0].astype(np.int64)
    j = nbrs[:, 1].astype(np.int64)
    E = i.shape[0]

    r_ij = (xyz[j] - xyz[i]).astype(np.float32)

    core = i // NPC
    iloc = i - core * NPC
    w = iloc // W
    irel = (iloc % W).astype(np.float32)
    key = core * NW + w
    order = np.argsort(key, kind="stable")
    cnt = np.bincount(key, minlength=N_CORES * NW).reshape(N_CORES, NW)
    n_t = np.maximum(1, -(-cnt.max(axis=0) // 128)).astype(int)
    TT = int(n_t.sum())
    starts = np.concatenate([[0], np.cumsum(n_t)]).astype(int)
    EPAD = TT * 128

    j_pad = np.zeros((N_CORES, EPAD), np.int64)
    irel_pad = np.full((N_CORES, EPAD), 200.0, np.float32)
    rij_pad = np.zeros((N_CORES, EPAD, 3), np.float32)

    cnt_flat = cnt.reshape(-1)
    grp_start = np.concatenate([[0], np.cumsum(cnt_flat)])[:-1]
    pos_in_grp = np.arange(E) - np.repeat(grp_start, cnt_flat)
    core_s = core[order]
    w_s = w[order]
    slot = starts[w_s] * 128 + pos_in_grp
    j_pad[core_s, slot] = j[order]
    irel_pad[core_s, slot] = irel[order]
    rij_pad[core_s, slot] = r_ij[order]

    # record-table row for node j: chunked-AllGather layout
    # (chunk c = (j%2048)//512 of rank r = j//2048 lands at 4096c + 512r)
    j_rec = (4096 * ((j_pad % SHARD) // 512) + 512 * (j_pad // SHARD)
             + (j_pad % 512))

    eidx = np.zeros((N_CORES, 128, TT * 8), np.int16)
    for wi in range(NW):
        nt = int(n_t[wi]); base = int(starts[wi])
        jw = j_rec[:, base * 128:(base + nt) * 128]
        c = np.arange(nt * 8)
        t, q = c // 8, c % 8
        r16 = np.arange(16)
        e_ix = t[None, :] * 128 + r16[:, None] + 16 * q[None, :]
        blk = jw[:, e_ix].astype(np.int16)
        eidx[:, :, base * 8:(base + nt) * 8] = np.tile(blk, (1, 8, 1))

    erel = irel_pad.reshape(N_CORES, TT, 128).transpose(0, 2, 1).copy()
    erij = rij_pad.reshape(N_CORES, TT, 128, 3).transpose(0, 2, 1, 3) \
        .reshape(N_CORES, 128, TT * 3).copy()

    alpha = np.asarray(alpha, np.float64)
    beta = np.asarray(beta, np.float64)
    W1 = np.asarray(W1, np.float64); W2 = np.asarray(W2, np.float64)
    W3 = np.asarray(W3, np.float64)
    b1 = np.asarray(b1, np.float64); b2 = np.asarray(b2, np.float64)
    b3 = np.asarray(b3, np.float64)
    assert np.all(np.abs(beta) > 1e-6), "beta==0 unsupported by silu fold"

    wmats, svcols = [], np.zeros((128, 30), np.float32)
    for idx in range(5):
        a, b = alpha[idx], beta[idx]
        wmats += [(a[0] / b[0])[:, None] * W1[idx],
                  (a[1] / b[1])[:, None] * W2[idx],
                  (a[2] / b[2])[:, None] * W3[idx]]
        svcols[:, idx * 6 + 0] = b[0]
        svcols[:, idx * 6 + 1] = b[1]
        svcols[:, idx * 6 + 2] = b[2]
        svcols[:, idx * 6 + 3] = b[1] * b1[idx]
        svcols[:, idx * 6 + 4] = b2[idx]
        svcols[:, idx * 6 + 5] = b3[idx]
    wst = np.stack(wmats).astype(np.float32).transpose(1, 0, 2) \
        .reshape(128, 15 * 128).copy()

    crow = np.zeros((128, 176), np.float32)
    crow[:, 0:128] = np.arange(128, dtype=np.float32)[None, :]
    ks = np.arange(16, dtype=np.float64)
    crow[:, 128:144] = (-GAMMA * ks)[None, :]
    crow[:, 144:160] = (15.0 - ks)[None, :]
    crow[:, 160:176] = np.log(_BINOM)[None, :]

    grep_np = np.zeros((128, 384), np.float32)
    for q in range(4):
        for X, G in enumerate([G_s, G_p, G_d]):
            grep_np[32 * q:32 * q + 16, X * 128:(X + 1) * 128] = \
                np.asarray(G, np.float32).T
    pdm_np = np.concatenate([np.asarray(M, np.float32).T for M in
                             (P_1, P_2, D_1, D_2)], axis=1).copy()

    xtT_np = np.zeros((128, NPAD), np.float32)
    xtT_np[:, :N_NODES] = x_tilde.T
    xtl_np = np.zeros((N_CORES, 128, LOCPAD), np.float32)
    for cix in range(N_CORES):
        xtl_np[cix, :, :NPC] = x_tilde[cix * NPC:(cix + 1) * NPC].T

    import ml_dtypes
    iota_bf = np.arange(128, dtype=np.float32)[None, :].repeat(128, 0)
    in_maps = []
    for cix in range(N_CORES):
        cstd = np.concatenate(
            [wst, svcols, crow, grep_np, pdm_np, xtl_np[cix], erel[cix]],
            axis=1).astype(np.float32)
        cstb = np.concatenate([iota_bf, erel[cix]], axis=1) \
            .astype(ml_dtypes.bfloat16)
        in_maps.append({
            "xtT": xtT_np[:, cix * SHARD:(cix + 1) * SHARD].copy(),
            "cstd": cstd, "cstb": cstb,
            "eidx": eidx[cix], "erij": erij[cix],
        })
    return tuple(int(x) for x in n_t), in_maps


def kernel(**inputs) -> np.ndarray:
    global LAST_EXEC_NS
    n_t, in_maps = _prep_host(**inputs)
    key = n_t + (os.environ.get("KB_GB", "0"), os.environ.get("KB_TPF", "0"))
    if key not in _prog_cache:
        _prog_cache[key] = _build_program(n_t)
    nc = _prog_cache[key]

    trace = os.environ.get("KBENCH_TRACE", "0") == "1"
    res = run_bass_kernel_spmd(nc, in_maps, core_ids=list(range(N_CORES)),
                               trace=trace)
    if trace:
        LAST_EXEC_NS = res.exec_time_ns
        global LAST_RESULT
        LAST_RESULT = res
        if res.instructions_and_trace:
            print(f"trace path: {res.instructions_and_trace[1]}")

    out = np.empty((N_NODES, FEAT), np.float32)
    for cix in range(N_CORES):
        out[cix * NPC:(cix + 1) * NPC] = res.results[cix]["outT"][:, :NPC].T
    return out
